# revision 37
# baseline (speedup 1.0000x reference)
"""Multi-head attention forward (B=2, N=2048, C=768, H=12) on 8 TRN2 cores.

Sharding: core = b*4 + g handles batch b, heads 3g..3g+2 (tensor parallel on
heads). Each core computes qkv for its heads, flash-style attention with the
full N x N logits kept on-chip (transposed [m, nq] layout so the key mask
folds into the exp bias and the softmax denominator comes from ones-columns
in V riding the PV matmul), and a partial output projection over its 192
channels. Host sums the 4 partials per batch and adds the bias.

All matmuls run in float32r (TF32-like, 1 cycle/row at free-dim >= 256).
K=64 logits matmuls for head pairs are row-packed onto disjoint PE row
groups (partitions 0:64 vs 64:128) so they run concurrently; head 2 gets
q/k duplicated into both partition halves so its two nq-halves pack the
same way. Head 1's V block is [ones | v] so its attention rows land on
psum partitions 64:128, making the whole normalize chain lane-local and
letting heads 0+1 share one K=128 projection matmul.
"""

import numpy as np

from concourse import bacc
import concourse.mybir as mybir
import concourse.tile as tile
from concourse.bass_utils import run_bass_kernel_spmd

B, N, C = 2, 2048, 768
H, DH = 12, 64
G = 4          # head groups (cores per batch)
HPC = 3        # heads per core
P = 128
KT = C // P    # 6 contraction tiles over channels
NMT = N // P   # 16 key (m) tiles
NQT = N // P   # 16 query tiles
W = 1024       # nq chunk width for logits/exp
NH = N // W    # 2 nq chunks
VBLK = 2 * DH  # 128: per-head lhsT width ([v|1] or [1|v])
VB = 5 * DH    # 320 per m-tile: [v0 | ones | v1 | ones | v2]

TRACE = False
LAST_EXEC_NS = None
LAST_RESULTS = None

_nc_cache = {}

f32 = mybir.dt.float32
f32r = mybir.dt.float32r

_VONES = np.ones((P, NMT * 2 * DH), np.float32)


def _build(reps=1):
    nc = bacc.Bacc("TRN2", debug=False)

    xT = nc.dram_tensor("xT", [C, N], f32r, kind="ExternalInput")
    wqkT = nc.dram_tensor("wqkT", [C, 6 * DH], f32r, kind="ExternalInput")
    wvT = nc.dram_tensor("wvT", [C, VB], f32r, kind="ExternalInput")
    wpT = nc.dram_tensor("wpT", [HPC * DH, C], f32r, kind="ExternalInput")
    mbias = nc.dram_tensor("mbias", [P, NMT], f32, kind="ExternalInput")
    vones = nc.dram_tensor("vones", [P, NMT * 2 * DH], f32r, kind="ExternalInput")
    y = nc.dram_tensor("y", [N, C], f32, kind="ExternalOutput")

    with tile.TileContext(nc) as tc:
        with (
            tc.tile_pool(name="big", bufs=1) as big,
            tc.tile_pool(name="exps", bufs=8) as exps,
            tc.tile_pool(name="recips", bufs=2) as recips,
            tc.tile_pool(name="ys", bufs=4) as ys,
            tc.tile_pool(name="pa", bufs=2, space="PSUM") as pa,
            tc.tile_pool(name="pb", bufs=2, space="PSUM") as pb,
        ):
            body(nc, tc, big, exps, recips, ys, pa, pb,
                 xT, wqkT, wvT, wpT, mbias, vones, y, reps)

    nc.compile()
    return nc


def body(nc, tc, big, exps, recips, ys, pa, pb,
         xT, wqkT, wvT, wpT, mbias, vones, y, reps):
    QC = HPC * DH  # 192: q block width in wqkT
    for _rep in range(reps):
        xT_sb = big.tile([P, KT * N], f32r, tag="xT", name="xT_sb")
        wqk_sb = big.tile([P, KT * 6 * DH], f32r, tag="wqk", name="wqk_sb")
        wv_sb = big.tile([P, KT * VB], f32r, tag="wv", name="wv_sb")
        wpA = big.tile([P, C], f32r, tag="wpA", name="wpA")   # heads 0+1
        wpB = big.tile([P, C], f32r, tag="wpB", name="wpB")   # head 2 rows 64:128
        mb_sb = big.tile([P, NMT], f32, tag="mb", name="mb_sb")
        ones_sb = big.tile([P, P], f32r, tag="ones", name="ones_sb")
        # t1: q/k for heads 0 (parts 0:64) and 1 (parts 64:128)
        # t2: q/k for head 2, duplicated into both partition halves
        t1 = big.tile([P, 2 * N], f32r, tag="t1", name="t1")
        t2 = big.tile([P, 2 * N], f32r, tag="t2", name="t2")
        v_sb = big.tile([P, NMT * VB], f32r, tag="v", name="v_sb")
        atA = big.tile([P, N], f32r, tag="atA", name="atA")   # h0 rows 0:64, h1 rows 64:128
        atB = big.tile([P, N], f32r, tag="atB", name="atB")   # h2 rows 64:128

        # --- input DMAs (coalesced; ordered by first use, xT first) ---
        nc.sync.dma_start(
            wqk_sb[:].rearrange("p (k c) -> p k c", c=6 * DH),
            wqkT[:, :].rearrange("(k p) c -> p k c", p=P),
        )
        xTv = xT[:, :].rearrange("(k p) n -> p k n", p=P)
        xsv = xT_sb[:].rearrange("p (k n) -> p k n", n=N)
        nc.sync.dma_start(xsv[:, 0:2, :], xTv[:, 0:2, :])
        nc.sync.dma_start(xsv[:, 2:4, :], xTv[:, 2:4, :])
        nc.sync.dma_start(xsv[:, 4:6, :], xTv[:, 4:6, :])
        nc.sync.dma_start(
            wv_sb[:].rearrange("p (k c) -> p k c", c=VB),
            wvT[:, :].rearrange("(k p) c -> p k c", p=P),
        )
        nc.sync.dma_start(mb_sb[:], mbias[:, :])
        nc.sync.dma_start(ones_sb[:], vones[:, 0:P])
        # ones regions of v_sb: cols 64:128 and 192:256 of each 320 block
        vview = v_sb[:].rearrange("p (m q) -> p m q", q=VB)
        nc.sync.dma_start(vview[:, :, DH : 2 * DH], vones[:, 0 : NMT * DH])
        nc.sync.dma_start(
            vview[:, :, 3 * DH : 4 * DH], vones[:, NMT * DH : 2 * NMT * DH]
        )
        nc.sync.dma_start(wpA[:], wpT[0 : 2 * DH, :])
        nc.sync.dma_start(wpB[DH:P, :], wpT[2 * DH : 3 * DH, :])

        # --- qT/kT in d-major layout ---
        def qk_pass(c0, copies, chunks=None):
            # copies: list of (psum row range, dest tile, dest col offset)
            for ch in chunks if chunks is not None else range(N // W):
                ps = pa.tile([P, W], f32, tag="pa", name="ps_qk")
                for s in range(W // 512):
                    for k in range(KT):
                        nc.tensor.matmul(
                            ps[:, s * 512 : (s + 1) * 512],
                            wqk_sb[:, k * 6 * DH + c0 : k * 6 * DH + c0 + P],
                            xT_sb[
                                :,
                                k * N + ch * W + s * 512 : k * N
                                + ch * W
                                + (s + 1) * 512,
                            ],
                            start=(k == 0),
                            stop=(k == KT - 1),
                        )
                for r0, r1, dest, dcol in copies:
                    nc.vector.tensor_copy(
                        dest[r0:r1, dcol + ch * W : dcol + (ch + 1) * W],
                        ps[r0:r1, :],
                    )


        # --- v in natural [m, d] layout ---
        def v_tile(mt, pool=None):
            ps = (pool or pb).tile([P, W], f32, tag="pa" if pool else "pb", name="ps_v")
            for k in range(KT):
                nc.tensor.matmul(
                    ps[:, :VB],
                    xT_sb[:, k * N + mt * P : k * N + (mt + 1) * P],
                    wv_sb[:, k * VB : (k + 1) * VB],
                    start=(k == 0),
                    stop=(k == KT - 1),
                )
            for c0 in (0, 2 * DH, 4 * DH):
                nc.vector.tensor_copy(
                    v_sb[:, mt * VB + c0 : mt * VB + c0 + DH],
                    ps[:, c0 : c0 + DH],
                )

        # per-head PV lhsT within m-tile block [v0 |1| v1 |1| v2]:
        # h0 = [v0|ones] (lo), h1 = [ones|v1] (hi), h2 = [ones|v2] (hi)
        def vap(h, mt):
            off = mt * VB + (0 if h == 0 else DH if h == 1 else 3 * DH)
            return v_sb[:, off : off + VBLK]

        # one softmax stream: two row-packed (lo/hi partition) attention
        # pipelines over the 16 key tiles, then lane-local normalize.
        # units: (qk tile, q col, v block col, out AP, flavor)
        # flavor 'lo': v block [v|1] -> attn rows 0:64, s row 64
        # flavor 'hi': v block [1|v] -> s row 0, attn rows 64:128
        def attn_stream(u_lo, u_hi, per_mt=None, tail=False, defer_norm=False):
            ps_lo = pb.tile([P, W], f32, tag="pb", name="ps_pv0")
            ps_hi = pb.tile([P, W], f32, tag="pb", name="ps_pv1")
            for mt in range(NMT):
                if per_mt is not None:
                    per_mt(mt)
                ets = []
                for (qk, qcol, vh, out_ap, flavor), prow in ((u_lo, 0), (u_hi, DH)):
                    ps_l = pa.tile([P, W], f32, tag="pa", name="ps_l")
                    for s in range(W // 512):
                        nc.tensor.matmul(
                            ps_l[:, s * 512 : (s + 1) * 512],
                            qk[prow : prow + DH, N + mt * P : N + (mt + 1) * P],
                            qk[prow : prow + DH, qcol + s * 512 : qcol + (s + 1) * 512],
                            start=True,
                            stop=True,
                        )
                    et = exps.tile([P, W], f32r, tag="exp", name="et")
                    nc.scalar.activation(
                        et[:],
                        ps_l[:],
                        mybir.ActivationFunctionType.Exp,
                        bias=mb_sb[:, mt : mt + 1],
                        scale=float(DH) ** -0.5,
                    )
                    ets.append(et)
                for et, ps_pv, (qk, qcol, vh, out_ap, flavor) in (
                    (ets[0], ps_lo, u_lo),
                    (ets[1], ps_hi, u_hi),
                ):
                    for s in range(W // 512):
                        nc.tensor.matmul(
                            ps_pv[:, s * 512 : (s + 1) * 512],
                            vap(vh, mt),
                            et[:, s * 512 : (s + 1) * 512],
                            start=(mt == 0),
                            stop=(mt == NMT - 1),
                        )
            # normalize (optionally deferred so the next stream's logits
            # claim psum slots before the norm's broadcast tile)
            def normalize():
              for ps_pv, (qk, qcol, vh, out_ap, flavor) in (
                (ps_lo, u_lo),
                (ps_hi, u_hi),
              ):
                rc = recips.tile([P, W], f32r, tag="rc", name="rc")
                ps_rb = pa.tile([P, W], f32, tag="pa", name="ps_rb")
                rb = recips.tile([P, W], f32, tag="rb", name="rb")
                if flavor == "lo":
                    srow, arow = DH, 0
                else:
                    srow, arow = 0, DH
                with nc.allow_low_precision(reason="f32r softmax denom"):
                    nc.vector.reciprocal(
                        rc[srow : srow + 1, :], ps_pv[srow : srow + 1, :]
                    )
                for s in range(W // 512):
                    nc.tensor.matmul(
                        ps_rb[:, s * 512 : (s + 1) * 512],
                        ones_sb[srow : srow + 1, :],
                        rc[srow : srow + 1, s * 512 : (s + 1) * 512],
                        start=True,
                        stop=True,
                    )
                if tail:
                    nc.scalar.copy(
                        rb[arow : arow + DH, :], ps_rb[arow : arow + DH, :]
                    )
                else:
                    nc.vector.tensor_copy(
                        rb[arow : arow + DH, :], ps_rb[arow : arow + DH, :]
                    )
                nc.vector.tensor_mul(
                    out_ap,
                    ps_pv[arow : arow + DH, :],
                    rb[arow : arow + DH, :],
                )

            if defer_norm:
                return normalize
            normalize()
            return None

        def proj(nt):
            ps_y = pa.tile([P, W], f32, tag="pa", name="ps_y")
            for o0, ow in ((0, 512), (512, 256)):
                nc.tensor.matmul(
                    ps_y[:, o0 : o0 + ow],
                    atA[:, nt * P : (nt + 1) * P],
                    wpA[:, o0 : o0 + ow],
                    start=True,
                    stop=False,
                )
                nc.tensor.matmul(
                    ps_y[:, o0 : o0 + ow],
                    atB[DH:P, nt * P : (nt + 1) * P],
                    wpB[DH:P, o0 : o0 + ow],
                    start=False,
                    stop=True,
                )
            yt = ys.tile([P, C], f32, tag="y", name="yt")
            if nt % 2 == 0:
                nc.vector.tensor_copy(yt[:], ps_y[:, :C])
            else:
                nc.scalar.copy(yt[:], ps_y[:, :C])
            nc.sync.dma_start(y[nt * P : (nt + 1) * P, :], yt[:])

        qk_pass(0, [(0, P, t1, 0)])    # q heads 0,1
        qk_pass(P, [(0, P, t1, N)])    # k heads 0,1
        for mt in range(NMT):
            v_tile(mt)
        # q2/k2: rows 0:64 = q2 -> t2 lo/q, rows 64:128 = k2 -> t2 hi/k;
        # the two SBUF->SBUF DMAs fill the other partition halves
        qk_pass(2 * P, [(0, DH, t2, 0), (DH, P, t2, N)])
        nc.sync.dma_start(t2[DH:P, 0:N], t2[0:DH, 0:N])      # q2 -> hi
        nc.sync.dma_start(t2[0:DH, N : 2 * N], t2[DH:P, N : 2 * N])  # k2 -> lo

        # heads 0+1, first nq half (normalize deferred into h2's stream)
        norm0 = attn_stream(
            (t1, 0, 0, atA[0:DH, 0:W], "lo"),
            (t1, 0, 1, atA[DH:P, 0:W], "hi"),
            defer_norm=True,
        )

        # head 2: both nq halves at once via the duplicated q2/k2 rows
        norm2 = attn_stream(
            (t2, 0, 2, atB[DH:P, 0:W], "hi"),
            (t2, W, 2, atB[DH:P, W : 2 * W], "hi"),
            per_mt=lambda mt: norm0() if mt == 2 else None,
            defer_norm=True,
        )

        # heads 0+1 second half: h2's deferred normalize fires early, then
        # half-0's projection tiles interleave so they overlap this stream
        def half1_hook(mt):
            if mt == 2:
                norm2()
            elif 4 <= mt < 4 + NQT // NH:
                proj(mt - 4)

        attn_stream(
            (t1, W, 0, atA[0:DH, W : 2 * W], "lo"),
            (t1, W, 1, atA[DH:P, W : 2 * W], "hi"),
            per_mt=half1_hook,
            tail=True,
        )
        for nt in range(NQT // NH, NQT):
            proj(nt)


def _get_nc(reps=1):
    if reps not in _nc_cache:
        _nc_cache[reps] = _build(reps)
    return _nc_cache[reps]


def kernel(x, att_mask, qkv_w, proj_w, proj_b):
    global LAST_EXEC_NS, LAST_RESULTS
    x = np.asarray(x, dtype=np.float32)
    att_mask = np.asarray(att_mask)
    qkv_w = np.asarray(qkv_w, dtype=np.float32)
    proj_w = np.asarray(proj_w, dtype=np.float32)
    proj_b = np.asarray(proj_b, dtype=np.float32)

    nc = _get_nc()

    in_maps = []
    for b in range(B):
        xT = np.ascontiguousarray(x[b].T)
        mb = np.where(att_mask[b] == 0, -1e30, 0.0).astype(np.float32)
        mbias = np.ascontiguousarray(mb.reshape(NMT, P).T)
        for g in range(G):
            r0 = g * HPC * DH
            r1 = (g + 1) * HPC * DH
            wq = qkv_w[r0:r1]                # [192, 768]
            wk = qkv_w[C + r0 : C + r1]
            wv = qkv_w[2 * C + r0 : 2 * C + r1]
            wqkT = np.ascontiguousarray(
                np.concatenate(
                    [wq[0 : 2 * DH], wk[0 : 2 * DH], wq[2 * DH :], wk[2 * DH :]], 0
                ).T
            )
            wvT = np.zeros((C, VB), np.float32)
            wvT[:, 0:DH] = wv[0:DH].T              # v h0 at block col 0
            wvT[:, 2 * DH : 3 * DH] = wv[DH : 2 * DH].T   # v h1 at 128:192
            wvT[:, 4 * DH : 5 * DH] = wv[2 * DH :].T      # v h2 at 256:320
            wpT = np.ascontiguousarray(proj_w[:, r0:r1].T)
            in_maps.append(
                {
                    "xT": xT,
                    "wqkT": wqkT,
                    "wvT": wvT,
                    "wpT": wpT,
                    "mbias": mbias,
                    "vones": _VONES,
                }
            )

    res = run_bass_kernel_spmd(
        nc, in_maps, core_ids=list(range(B * G)), trace=TRACE
    )
    LAST_EXEC_NS = res.exec_time_ns
    LAST_RESULTS = res

    out = np.zeros((B, N, C), np.float32)
    for b in range(B):
        acc = res.results[b * G]["y"].copy()
        for g in range(1, G):
            acc += res.results[b * G + g]["y"]
        out[b] = acc + proj_b[None, :]
    return out


# revision 43
# speedup vs baseline: 1.0008x; 1.0008x over previous
"""Multi-head attention forward (B=2, N=2048, C=768, H=12) on 8 TRN2 cores.

Sharding: core = b*4 + g handles batch b, heads 3g..3g+2 (tensor parallel on
heads). Each core computes qkv for its heads, flash-style attention with the
full N x N logits kept on-chip (transposed [m, nq] layout so the key mask
folds into the exp bias and the softmax denominator comes from ones-columns
in V riding the PV matmul), and a partial output projection over its 192
channels. Host sums the 4 partials per batch and adds the bias.

All matmuls run in float32r (TF32-like, 1 cycle/row at free-dim >= 256).
K=64 logits matmuls for head pairs are row-packed onto disjoint PE row
groups (partitions 0:64 vs 64:128) so they run concurrently; head 2 gets
q/k duplicated into both partition halves so its two nq-halves pack the
same way. Head 1's V block is [ones | v] so its attention rows land on
psum partitions 64:128, making the whole normalize chain lane-local and
letting heads 0+1 share one K=128 projection matmul.
"""

import numpy as np

from concourse import bacc
import concourse.mybir as mybir
import concourse.tile as tile
from concourse.bass_utils import run_bass_kernel_spmd

B, N, C = 2, 2048, 768
H, DH = 12, 64
G = 4          # head groups (cores per batch)
HPC = 3        # heads per core
P = 128
KT = C // P    # 6 contraction tiles over channels
NMT = N // P   # 16 key (m) tiles
NQT = N // P   # 16 query tiles
W = 1024       # nq chunk width for logits/exp
NH = N // W    # 2 nq chunks
VBLK = 2 * DH  # 128: per-head lhsT width ([v|1] or [1|v])
VB = 5 * DH    # 320 per m-tile: [v0 | ones | v1 | ones | v2]

TRACE = False
LAST_EXEC_NS = None
LAST_RESULTS = None

_nc_cache = {}

f32 = mybir.dt.float32
f32r = mybir.dt.float32r

_VONES = np.ones((P, NMT * 2 * DH), np.float32)


def _build(reps=1):
    nc = bacc.Bacc("TRN2", debug=False)

    xT = nc.dram_tensor("xT", [C, N], f32r, kind="ExternalInput")
    wqkT = nc.dram_tensor("wqkT", [C, 6 * DH], f32r, kind="ExternalInput")
    wvT = nc.dram_tensor("wvT", [C, VB], f32r, kind="ExternalInput")
    wpT = nc.dram_tensor("wpT", [HPC * DH, C], f32r, kind="ExternalInput")
    mbias = nc.dram_tensor("mbias", [P, NMT], f32, kind="ExternalInput")
    vones = nc.dram_tensor("vones", [P, NMT * 2 * DH], f32r, kind="ExternalInput")
    y = nc.dram_tensor("y", [N, C], f32, kind="ExternalOutput")

    with tile.TileContext(nc) as tc:
        with (
            tc.tile_pool(name="big", bufs=1) as big,
            tc.tile_pool(name="exps", bufs=8) as exps,
            tc.tile_pool(name="recips", bufs=2) as recips,
            tc.tile_pool(name="ys", bufs=6) as ys,
            tc.tile_pool(name="pa", bufs=2, space="PSUM") as pa,
            tc.tile_pool(name="pb", bufs=2, space="PSUM") as pb,
        ):
            body(nc, tc, big, exps, recips, ys, pa, pb,
                 xT, wqkT, wvT, wpT, mbias, vones, y, reps)

    nc.compile()
    return nc


def body(nc, tc, big, exps, recips, ys, pa, pb,
         xT, wqkT, wvT, wpT, mbias, vones, y, reps):
    QC = HPC * DH  # 192: q block width in wqkT
    for _rep in range(reps):
        xT_sb = big.tile([P, KT * N], f32r, tag="xT", name="xT_sb")
        wqk_sb = big.tile([P, KT * 6 * DH], f32r, tag="wqk", name="wqk_sb")
        wv_sb = big.tile([P, KT * VB], f32r, tag="wv", name="wv_sb")
        wpA = big.tile([P, C], f32r, tag="wpA", name="wpA")   # heads 0+1
        wpB = big.tile([P, C], f32r, tag="wpB", name="wpB")   # head 2 rows 64:128
        mb_sb = big.tile([P, NMT], f32, tag="mb", name="mb_sb")
        ones_sb = big.tile([P, P], f32r, tag="ones", name="ones_sb")
        # t1: q/k for heads 0 (parts 0:64) and 1 (parts 64:128)
        # t2: q/k for head 2, duplicated into both partition halves
        t1 = big.tile([P, 2 * N], f32r, tag="t1", name="t1")
        t2 = big.tile([P, 2 * N], f32r, tag="t2", name="t2")
        v_sb = big.tile([P, NMT * VB], f32r, tag="v", name="v_sb")
        atA = big.tile([P, N], f32r, tag="atA", name="atA")   # h0 rows 0:64, h1 rows 64:128
        atB = big.tile([P, N], f32r, tag="atB", name="atB")   # h2 rows 64:128

        # --- input DMAs (coalesced; ordered by first use, xT first) ---
        nc.sync.dma_start(
            wqk_sb[:].rearrange("p (k c) -> p k c", c=6 * DH),
            wqkT[:, :].rearrange("(k p) c -> p k c", p=P),
        )
        xTv = xT[:, :].rearrange("(k p) n -> p k n", p=P)
        xsv = xT_sb[:].rearrange("p (k n) -> p k n", n=N)
        nc.sync.dma_start(xsv[:, 0:2, :], xTv[:, 0:2, :])
        nc.sync.dma_start(xsv[:, 2:4, :], xTv[:, 2:4, :])
        nc.sync.dma_start(xsv[:, 4:6, :], xTv[:, 4:6, :])
        nc.sync.dma_start(
            wv_sb[:].rearrange("p (k c) -> p k c", c=VB),
            wvT[:, :].rearrange("(k p) c -> p k c", p=P),
        )
        nc.sync.dma_start(mb_sb[:], mbias[:, :])
        nc.sync.dma_start(ones_sb[:], vones[:, 0:P])
        # ones regions of v_sb: cols 64:128 and 192:256 of each 320 block
        vview = v_sb[:].rearrange("p (m q) -> p m q", q=VB)
        nc.sync.dma_start(vview[:, :, DH : 2 * DH], vones[:, 0 : NMT * DH])
        nc.sync.dma_start(
            vview[:, :, 3 * DH : 4 * DH], vones[:, NMT * DH : 2 * NMT * DH]
        )
        nc.sync.dma_start(wpA[:], wpT[0 : 2 * DH, :])
        nc.sync.dma_start(wpB[DH:P, :], wpT[2 * DH : 3 * DH, :])

        # --- qT/kT in d-major layout ---
        def qk_pass(c0, copies, chunks=None):
            # copies: list of (psum row range, dest tile, dest col offset)
            for ch in chunks if chunks is not None else range(N // W):
                ps = pa.tile([P, W], f32, tag="pa", name="ps_qk")
                for s in range(W // 512):
                    for k in range(KT):
                        nc.tensor.matmul(
                            ps[:, s * 512 : (s + 1) * 512],
                            wqk_sb[:, k * 6 * DH + c0 : k * 6 * DH + c0 + P],
                            xT_sb[
                                :,
                                k * N + ch * W + s * 512 : k * N
                                + ch * W
                                + (s + 1) * 512,
                            ],
                            start=(k == 0),
                            stop=(k == KT - 1),
                        )
                for r0, r1, dest, dcol in copies:
                    nc.vector.tensor_copy(
                        dest[r0:r1, dcol + ch * W : dcol + (ch + 1) * W],
                        ps[r0:r1, :],
                    )


        # --- v in natural [m, d] layout ---
        def v_tile(mt, pool=None):
            ps = (pool or pb).tile([P, W], f32, tag="pa" if pool else "pb", name="ps_v")
            for k in range(KT):
                nc.tensor.matmul(
                    ps[:, :VB],
                    xT_sb[:, k * N + mt * P : k * N + (mt + 1) * P],
                    wv_sb[:, k * VB : (k + 1) * VB],
                    start=(k == 0),
                    stop=(k == KT - 1),
                )
            for c0 in (0, 2 * DH, 4 * DH):
                nc.vector.tensor_copy(
                    v_sb[:, mt * VB + c0 : mt * VB + c0 + DH],
                    ps[:, c0 : c0 + DH],
                )

        # per-head PV lhsT within m-tile block [v0 |1| v1 |1| v2]:
        # h0 = [v0|ones] (lo), h1 = [ones|v1] (hi), h2 = [ones|v2] (hi)
        def vap(h, mt):
            off = mt * VB + (0 if h == 0 else DH if h == 1 else 3 * DH)
            return v_sb[:, off : off + VBLK]

        # one softmax stream: two row-packed (lo/hi partition) attention
        # pipelines over the 16 key tiles, then lane-local normalize.
        # units: (qk tile, q col, v block col, out AP, flavor)
        # flavor 'lo': v block [v|1] -> attn rows 0:64, s row 64
        # flavor 'hi': v block [1|v] -> s row 0, attn rows 64:128
        def attn_stream(u_lo, u_hi, per_mt=None, tail=False, defer_norm=False):
            ps_lo = pb.tile([P, W], f32, tag="pb", name="ps_pv0")
            ps_hi = pb.tile([P, W], f32, tag="pb", name="ps_pv1")
            for mt in range(NMT):
                if per_mt is not None:
                    per_mt(mt)
                ets = []
                for (qk, qcol, vh, out_ap, flavor), prow in ((u_lo, 0), (u_hi, DH)):
                    ps_l = pa.tile([P, W], f32, tag="pa", name="ps_l")
                    for s in range(W // 512):
                        nc.tensor.matmul(
                            ps_l[:, s * 512 : (s + 1) * 512],
                            qk[prow : prow + DH, N + mt * P : N + (mt + 1) * P],
                            qk[prow : prow + DH, qcol + s * 512 : qcol + (s + 1) * 512],
                            start=True,
                            stop=True,
                        )
                    et = exps.tile([P, W], f32r, tag="exp", name="et")
                    nc.scalar.activation(
                        et[:],
                        ps_l[:],
                        mybir.ActivationFunctionType.Exp,
                        bias=mb_sb[:, mt : mt + 1],
                        scale=float(DH) ** -0.5,
                    )
                    ets.append(et)
                for et, ps_pv, (qk, qcol, vh, out_ap, flavor) in (
                    (ets[0], ps_lo, u_lo),
                    (ets[1], ps_hi, u_hi),
                ):
                    for s in range(W // 512):
                        nc.tensor.matmul(
                            ps_pv[:, s * 512 : (s + 1) * 512],
                            vap(vh, mt),
                            et[:, s * 512 : (s + 1) * 512],
                            start=(mt == 0),
                            stop=(mt == NMT - 1),
                        )
            # normalize (optionally deferred so the next stream's logits
            # claim psum slots before the norm's broadcast tile)
            def normalize():
              for ps_pv, (qk, qcol, vh, out_ap, flavor) in (
                (ps_lo, u_lo),
                (ps_hi, u_hi),
              ):
                rc = recips.tile([P, W], f32r, tag="rc", name="rc")
                ps_rb = pa.tile([P, W], f32, tag="pa", name="ps_rb")
                rb = recips.tile([P, W], f32, tag="rb", name="rb")
                if flavor == "lo":
                    srow, arow = DH, 0
                else:
                    srow, arow = 0, DH
                with nc.allow_low_precision(reason="f32r softmax denom"):
                    nc.vector.reciprocal(
                        rc[srow : srow + 1, :], ps_pv[srow : srow + 1, :]
                    )
                for s in range(W // 512):
                    nc.tensor.matmul(
                        ps_rb[:, s * 512 : (s + 1) * 512],
                        ones_sb[srow : srow + 1, :],
                        rc[srow : srow + 1, s * 512 : (s + 1) * 512],
                        start=True,
                        stop=True,
                    )
                if tail:
                    nc.scalar.copy(
                        rb[arow : arow + DH, :], ps_rb[arow : arow + DH, :]
                    )
                else:
                    nc.vector.tensor_copy(
                        rb[arow : arow + DH, :], ps_rb[arow : arow + DH, :]
                    )
                nc.vector.tensor_mul(
                    out_ap,
                    ps_pv[arow : arow + DH, :],
                    rb[arow : arow + DH, :],
                )

            if defer_norm:
                return normalize
            normalize()
            return None

        def proj(nt):
            ps_y = pa.tile([P, W], f32, tag="pa", name="ps_y")
            for o0, ow in ((0, 512), (512, 256)):
                nc.tensor.matmul(
                    ps_y[:, o0 : o0 + ow],
                    atA[:, nt * P : (nt + 1) * P],
                    wpA[:, o0 : o0 + ow],
                    start=True,
                    stop=False,
                )
                nc.tensor.matmul(
                    ps_y[:, o0 : o0 + ow],
                    atB[DH:P, nt * P : (nt + 1) * P],
                    wpB[DH:P, o0 : o0 + ow],
                    start=False,
                    stop=True,
                )
            yt = ys.tile([P, C], f32, tag="y", name="yt")
            if nt % 2 == 0:
                nc.vector.tensor_copy(yt[:], ps_y[:, :C])
            else:
                nc.scalar.copy(yt[:], ps_y[:, :C])
            nc.sync.dma_start(y[nt * P : (nt + 1) * P, :], yt[:])

        qk_pass(0, [(0, P, t1, 0)])    # q heads 0,1
        qk_pass(P, [(0, P, t1, N)])    # k heads 0,1
        for mt in range(NMT):
            v_tile(mt)
        # q2/k2: rows 0:64 = q2 -> t2 lo/q, rows 64:128 = k2 -> t2 hi/k;
        # the two SBUF->SBUF DMAs fill the other partition halves
        qk_pass(2 * P, [(0, DH, t2, 0), (DH, P, t2, N)])
        nc.sync.dma_start(t2[DH:P, 0:N], t2[0:DH, 0:N])      # q2 -> hi
        nc.sync.dma_start(t2[0:DH, N : 2 * N], t2[DH:P, N : 2 * N])  # k2 -> lo

        # heads 0+1, first nq half (normalize deferred into h2's stream)
        norm0 = attn_stream(
            (t1, 0, 0, atA[0:DH, 0:W], "lo"),
            (t1, 0, 1, atA[DH:P, 0:W], "hi"),
            defer_norm=True,
        )

        # head 2: both nq halves at once via the duplicated q2/k2 rows
        norm2 = attn_stream(
            (t2, 0, 2, atB[DH:P, 0:W], "hi"),
            (t2, W, 2, atB[DH:P, W : 2 * W], "hi"),
            per_mt=lambda mt: norm0() if mt == 2 else None,
            defer_norm=True,
        )

        # heads 0+1 second half: h2's deferred normalize fires early, then
        # half-0's projection tiles interleave so they overlap this stream
        def half1_hook(mt):
            if mt == 2:
                norm2()
            elif 4 <= mt < 4 + NQT // NH:
                proj(mt - 4)

        attn_stream(
            (t1, W, 0, atA[0:DH, W : 2 * W], "lo"),
            (t1, W, 1, atA[DH:P, W : 2 * W], "hi"),
            per_mt=half1_hook,
            tail=True,
        )
        for nt in range(NQT // NH, NQT):
            proj(nt)


def _get_nc(reps=1):
    if reps not in _nc_cache:
        _nc_cache[reps] = _build(reps)
    return _nc_cache[reps]


def kernel(x, att_mask, qkv_w, proj_w, proj_b):
    global LAST_EXEC_NS, LAST_RESULTS
    x = np.asarray(x, dtype=np.float32)
    att_mask = np.asarray(att_mask)
    qkv_w = np.asarray(qkv_w, dtype=np.float32)
    proj_w = np.asarray(proj_w, dtype=np.float32)
    proj_b = np.asarray(proj_b, dtype=np.float32)

    nc = _get_nc()

    in_maps = []
    for b in range(B):
        xT = np.ascontiguousarray(x[b].T)
        mb = np.where(att_mask[b] == 0, -1e30, 0.0).astype(np.float32)
        mbias = np.ascontiguousarray(mb.reshape(NMT, P).T)
        for g in range(G):
            r0 = g * HPC * DH
            r1 = (g + 1) * HPC * DH
            wq = qkv_w[r0:r1]                # [192, 768]
            wk = qkv_w[C + r0 : C + r1]
            wv = qkv_w[2 * C + r0 : 2 * C + r1]
            wqkT = np.ascontiguousarray(
                np.concatenate(
                    [wq[0 : 2 * DH], wk[0 : 2 * DH], wq[2 * DH :], wk[2 * DH :]], 0
                ).T
            )
            wvT = np.zeros((C, VB), np.float32)
            wvT[:, 0:DH] = wv[0:DH].T              # v h0 at block col 0
            wvT[:, 2 * DH : 3 * DH] = wv[DH : 2 * DH].T   # v h1 at 128:192
            wvT[:, 4 * DH : 5 * DH] = wv[2 * DH :].T      # v h2 at 256:320
            wpT = np.ascontiguousarray(proj_w[:, r0:r1].T)
            in_maps.append(
                {
                    "xT": xT,
                    "wqkT": wqkT,
                    "wvT": wvT,
                    "wpT": wpT,
                    "mbias": mbias,
                    "vones": _VONES,
                }
            )

    res = run_bass_kernel_spmd(
        nc, in_maps, core_ids=list(range(B * G)), trace=TRACE
    )
    LAST_EXEC_NS = res.exec_time_ns
    LAST_RESULTS = res

    out = np.zeros((B, N, C), np.float32)
    for b in range(B):
        acc = res.results[b * G]["y"].copy()
        for g in range(1, G):
            acc += res.results[b * G + g]["y"]
        out[b] = acc + proj_b[None, :]
    return out


# revision 44
# speedup vs baseline: 1.0180x; 1.0172x over previous
"""Multi-head attention forward (B=2, N=2048, C=768, H=12) on 8 TRN2 cores.

Sharding: core = b*4 + g handles batch b, heads 3g..3g+2 (tensor parallel on
heads). Each core computes qkv for its heads, flash-style attention with the
full N x N logits kept on-chip (transposed [m, nq] layout so the key mask
folds into the exp bias and the softmax denominator comes from ones-columns
in V riding the PV matmul), and a partial output projection over its 192
channels. Host sums the 4 partials per batch and adds the bias.

All matmuls run in float32r (TF32-like, 1 cycle/row at free-dim >= 256).
K=64 logits matmuls for head pairs are row-packed onto disjoint PE row
groups (partitions 0:64 vs 64:128) so they run concurrently; head 2 gets
q/k duplicated into both partition halves so its two nq-halves pack the
same way. Head 1's V block is [ones | v] so its attention rows land on
psum partitions 64:128, making the whole normalize chain lane-local and
letting heads 0+1 share one K=128 projection matmul.
"""

import numpy as np

from concourse import bacc
import concourse.mybir as mybir
import concourse.tile as tile
from concourse.bass_utils import run_bass_kernel_spmd

B, N, C = 2, 2048, 768
H, DH = 12, 64
G = 4          # head groups (cores per batch)
HPC = 3        # heads per core
P = 128
KT = C // P    # 6 contraction tiles over channels
NMT = N // P   # 16 key (m) tiles
NQT = N // P   # 16 query tiles
W = 1024       # nq chunk width for logits/exp
NH = N // W    # 2 nq chunks
VBLK = 2 * DH  # 128: per-head lhsT width ([v|1] or [1|v])
VB = 5 * DH    # 320 per m-tile: [v0 | ones | v1 | ones | v2]

TRACE = False
LAST_EXEC_NS = None
LAST_RESULTS = None

_nc_cache = {}

f32 = mybir.dt.float32
f32r = mybir.dt.float32r

_VONES = np.ones((P, NMT * 2 * DH), np.float32)


def _build(reps=1):
    nc = bacc.Bacc("TRN2", debug=False)

    xT = nc.dram_tensor("xT", [C, N], f32r, kind="ExternalInput")
    wqkT = nc.dram_tensor("wqkT", [C, 6 * DH], f32r, kind="ExternalInput")
    wvT = nc.dram_tensor("wvT", [C, VB], f32r, kind="ExternalInput")
    wpT = nc.dram_tensor("wpT", [HPC * DH, C], f32r, kind="ExternalInput")
    mbias = nc.dram_tensor("mbias", [P, NMT], f32, kind="ExternalInput")
    vones = nc.dram_tensor("vones", [P, NMT * 2 * DH], f32r, kind="ExternalInput")
    y = nc.dram_tensor("y", [N, C], f32, kind="ExternalOutput")

    with tile.TileContext(nc) as tc:
        with (
            tc.tile_pool(name="big", bufs=1) as big,
            tc.tile_pool(name="exps", bufs=8) as exps,
            tc.tile_pool(name="recips", bufs=2) as recips,
            tc.tile_pool(name="ys", bufs=6) as ys,
            tc.tile_pool(name="pa", bufs=2, space="PSUM") as pa,
            tc.tile_pool(name="pb", bufs=2, space="PSUM") as pb,
        ):
            body(nc, tc, big, exps, recips, ys, pa, pb,
                 xT, wqkT, wvT, wpT, mbias, vones, y, reps)

    nc.compile()
    return nc


def body(nc, tc, big, exps, recips, ys, pa, pb,
         xT, wqkT, wvT, wpT, mbias, vones, y, reps):
    QC = HPC * DH  # 192: q block width in wqkT
    for _rep in range(reps):
        xT_sb = big.tile([P, KT * N], f32r, tag="xT", name="xT_sb")
        wqk_sb = big.tile([P, KT * 6 * DH], f32r, tag="wqk", name="wqk_sb")
        wv_sb = big.tile([P, KT * VB], f32r, tag="wv", name="wv_sb")
        wpA = big.tile([P, C], f32r, tag="wpA", name="wpA")   # heads 0+1
        wpB = big.tile([P, C], f32r, tag="wpB", name="wpB")   # head 2 rows 64:128
        mb_sb = big.tile([P, NMT], f32, tag="mb", name="mb_sb")
        ones_sb = big.tile([P, P], f32r, tag="ones", name="ones_sb")
        # t1: q/k for heads 0 (parts 0:64) and 1 (parts 64:128)
        # t2: q/k for head 2, duplicated into both partition halves
        t1 = big.tile([P, 2 * N], f32r, tag="t1", name="t1")
        t2 = big.tile([P, 2 * N], f32r, tag="t2", name="t2")
        v_sb = big.tile([P, NMT * VB], f32r, tag="v", name="v_sb")
        atA = big.tile([P, N], f32r, tag="atA", name="atA")   # h0 rows 0:64, h1 rows 64:128
        atB = big.tile([P, N], f32r, tag="atB", name="atB")   # h2 rows 64:128

        # --- input DMAs (coalesced; ordered by first use, xT first) ---
        nc.sync.dma_start(
            wqk_sb[:].rearrange("p (k c) -> p k c", c=6 * DH),
            wqkT[:, :].rearrange("(k p) c -> p k c", p=P),
        )
        xTv = xT[:, :].rearrange("(k p) n -> p k n", p=P)
        xsv = xT_sb[:].rearrange("p (k n) -> p k n", n=N)
        nc.sync.dma_start(xsv[:, 0:2, :], xTv[:, 0:2, :])
        nc.sync.dma_start(xsv[:, 2:4, :], xTv[:, 2:4, :])
        nc.sync.dma_start(xsv[:, 4:6, :], xTv[:, 4:6, :])
        nc.sync.dma_start(
            wv_sb[:].rearrange("p (k c) -> p k c", c=VB),
            wvT[:, :].rearrange("(k p) c -> p k c", p=P),
        )
        nc.sync.dma_start(mb_sb[:], mbias[:, :])
        nc.sync.dma_start(ones_sb[:], vones[:, 0:P])
        # ones regions of v_sb: cols 64:128 and 192:256 of each 320 block
        vview = v_sb[:].rearrange("p (m q) -> p m q", q=VB)
        nc.sync.dma_start(vview[:, :, DH : 2 * DH], vones[:, 0 : NMT * DH])
        nc.sync.dma_start(
            vview[:, :, 3 * DH : 4 * DH], vones[:, NMT * DH : 2 * NMT * DH]
        )
        nc.sync.dma_start(wpA[:], wpT[0 : 2 * DH, :])
        nc.sync.dma_start(wpB[DH:P, :], wpT[2 * DH : 3 * DH, :])

        # --- qT/kT in d-major layout ---
        def qk_pass(c0, copies, chunks=None, pool=None):
            # copies: list of (psum row range, dest tile, dest col offset)
            for ch in chunks if chunks is not None else range(N // W):
                ps = (pool or pa).tile(
                    [P, W], f32, tag="pb" if pool else "pa", name="ps_qk"
                )
                for s in range(W // 512):
                    for k in range(KT):
                        nc.tensor.matmul(
                            ps[:, s * 512 : (s + 1) * 512],
                            wqk_sb[:, k * 6 * DH + c0 : k * 6 * DH + c0 + P],
                            xT_sb[
                                :,
                                k * N + ch * W + s * 512 : k * N
                                + ch * W
                                + (s + 1) * 512,
                            ],
                            start=(k == 0),
                            stop=(k == KT - 1),
                        )
                for r0, r1, dest, dcol in copies:
                    nc.vector.tensor_copy(
                        dest[r0:r1, dcol + ch * W : dcol + (ch + 1) * W],
                        ps[r0:r1, :],
                    )


        # --- v in natural [m, d] layout ---
        def v_tile(mt, pool=None):
            ps = (pool or pb).tile([P, W], f32, tag="pa" if pool else "pb", name="ps_v")
            for k in range(KT):
                nc.tensor.matmul(
                    ps[:, :VB],
                    xT_sb[:, k * N + mt * P : k * N + (mt + 1) * P],
                    wv_sb[:, k * VB : (k + 1) * VB],
                    start=(k == 0),
                    stop=(k == KT - 1),
                )
            for c0 in (0, 2 * DH, 4 * DH):
                nc.vector.tensor_copy(
                    v_sb[:, mt * VB + c0 : mt * VB + c0 + DH],
                    ps[:, c0 : c0 + DH],
                )

        # per-head PV lhsT within m-tile block [v0 |1| v1 |1| v2]:
        # h0 = [v0|ones] (lo), h1 = [ones|v1] (hi), h2 = [ones|v2] (hi)
        def vap(h, mt):
            off = mt * VB + (0 if h == 0 else DH if h == 1 else 3 * DH)
            return v_sb[:, off : off + VBLK]

        # one softmax stream: two row-packed (lo/hi partition) attention
        # pipelines over the 16 key tiles, then lane-local normalize.
        # units: (qk tile, q col, v block col, out AP, flavor)
        # flavor 'lo': v block [v|1] -> attn rows 0:64, s row 64
        # flavor 'hi': v block [1|v] -> s row 0, attn rows 64:128
        def attn_stream(u_lo, u_hi, per_mt=None, tail=False, defer_norm=False):
            ps_lo = pb.tile([P, W], f32, tag="pb", name="ps_pv0")
            ps_hi = pb.tile([P, W], f32, tag="pb", name="ps_pv1")
            for mt in range(NMT):
                if per_mt is not None:
                    per_mt(mt)
                ets = []
                for (qk, qcol, vh, out_ap, flavor), prow in ((u_lo, 0), (u_hi, DH)):
                    ps_l = pa.tile([P, W], f32, tag="pa", name="ps_l")
                    for s in range(W // 512):
                        nc.tensor.matmul(
                            ps_l[:, s * 512 : (s + 1) * 512],
                            qk[prow : prow + DH, N + mt * P : N + (mt + 1) * P],
                            qk[prow : prow + DH, qcol + s * 512 : qcol + (s + 1) * 512],
                            start=True,
                            stop=True,
                        )
                    et = exps.tile([P, W], f32r, tag="exp", name="et")
                    nc.scalar.activation(
                        et[:],
                        ps_l[:],
                        mybir.ActivationFunctionType.Exp,
                        bias=mb_sb[:, mt : mt + 1],
                        scale=float(DH) ** -0.5,
                    )
                    ets.append(et)
                for et, ps_pv, (qk, qcol, vh, out_ap, flavor) in (
                    (ets[0], ps_lo, u_lo),
                    (ets[1], ps_hi, u_hi),
                ):
                    for s in range(W // 512):
                        nc.tensor.matmul(
                            ps_pv[:, s * 512 : (s + 1) * 512],
                            vap(vh, mt),
                            et[:, s * 512 : (s + 1) * 512],
                            start=(mt == 0),
                            stop=(mt == NMT - 1),
                        )
            # normalize (optionally deferred so the next stream's logits
            # claim psum slots before the norm's broadcast tile)
            def normalize():
              for ps_pv, (qk, qcol, vh, out_ap, flavor) in (
                (ps_lo, u_lo),
                (ps_hi, u_hi),
              ):
                rc = recips.tile([P, W], f32r, tag="rc", name="rc")
                ps_rb = pa.tile([P, W], f32, tag="pa", name="ps_rb")
                rb = recips.tile([P, W], f32, tag="rb", name="rb")
                if flavor == "lo":
                    srow, arow = DH, 0
                else:
                    srow, arow = 0, DH
                with nc.allow_low_precision(reason="f32r softmax denom"):
                    nc.vector.reciprocal(
                        rc[srow : srow + 1, :], ps_pv[srow : srow + 1, :]
                    )
                for s in range(W // 512):
                    nc.tensor.matmul(
                        ps_rb[:, s * 512 : (s + 1) * 512],
                        ones_sb[srow : srow + 1, :],
                        rc[srow : srow + 1, s * 512 : (s + 1) * 512],
                        start=True,
                        stop=True,
                    )
                if tail:
                    nc.scalar.copy(
                        rb[arow : arow + DH, :], ps_rb[arow : arow + DH, :]
                    )
                else:
                    nc.vector.tensor_copy(
                        rb[arow : arow + DH, :], ps_rb[arow : arow + DH, :]
                    )
                nc.vector.tensor_mul(
                    out_ap,
                    ps_pv[arow : arow + DH, :],
                    rb[arow : arow + DH, :],
                )

            if defer_norm:
                return normalize
            normalize()
            return None

        def proj(nt):
            ps_y = pa.tile([P, W], f32, tag="pa", name="ps_y")
            for o0, ow in ((0, 512), (512, 256)):
                nc.tensor.matmul(
                    ps_y[:, o0 : o0 + ow],
                    atA[:, nt * P : (nt + 1) * P],
                    wpA[:, o0 : o0 + ow],
                    start=True,
                    stop=False,
                )
                nc.tensor.matmul(
                    ps_y[:, o0 : o0 + ow],
                    atB[DH:P, nt * P : (nt + 1) * P],
                    wpB[DH:P, o0 : o0 + ow],
                    start=False,
                    stop=True,
                )
            yt = ys.tile([P, C], f32, tag="y", name="yt")
            if nt % 2 == 0:
                nc.vector.tensor_copy(yt[:], ps_y[:, :C])
            else:
                nc.scalar.copy(yt[:], ps_y[:, :C])
            nc.sync.dma_start(y[nt * P : (nt + 1) * P, :], yt[:])

        qk_pass(0, [(0, P, t1, 0)])    # q heads 0,1
        qk_pass(P, [(0, P, t1, N)])    # k heads 0,1
        for mt in range(NMT):
            v_tile(mt)
        # q2/k2: rows 0:64 = q2 -> t2 lo/q, rows 64:128 = k2 -> t2 hi/k;
        # the two SBUF->SBUF DMAs fill the other partition halves
        qk_pass(2 * P, [(0, DH, t2, 0), (DH, P, t2, N)], pool=pb)
        nc.sync.dma_start(t2[DH:P, 0:N], t2[0:DH, 0:N])      # q2 -> hi
        nc.sync.dma_start(t2[0:DH, N : 2 * N], t2[DH:P, N : 2 * N])  # k2 -> lo

        # heads 0+1, first nq half (normalize deferred into h2's stream)
        norm0 = attn_stream(
            (t1, 0, 0, atA[0:DH, 0:W], "lo"),
            (t1, 0, 1, atA[DH:P, 0:W], "hi"),
            defer_norm=True,
        )

        # head 2: both nq halves at once via the duplicated q2/k2 rows
        norm2 = attn_stream(
            (t2, 0, 2, atB[DH:P, 0:W], "hi"),
            (t2, W, 2, atB[DH:P, W : 2 * W], "hi"),
            per_mt=lambda mt: norm0() if mt == 2 else None,
            defer_norm=True,
        )

        # heads 0+1 second half: h2's deferred normalize fires early, then
        # half-0's projection tiles interleave so they overlap this stream
        def half1_hook(mt):
            if mt == 2:
                norm2()
            elif 4 <= mt < 4 + NQT // NH:
                proj(mt - 4)

        attn_stream(
            (t1, W, 0, atA[0:DH, W : 2 * W], "lo"),
            (t1, W, 1, atA[DH:P, W : 2 * W], "hi"),
            per_mt=half1_hook,
            tail=True,
        )
        for nt in range(NQT // NH, NQT):
            proj(nt)


def _get_nc(reps=1):
    if reps not in _nc_cache:
        _nc_cache[reps] = _build(reps)
    return _nc_cache[reps]


def kernel(x, att_mask, qkv_w, proj_w, proj_b):
    global LAST_EXEC_NS, LAST_RESULTS
    x = np.asarray(x, dtype=np.float32)
    att_mask = np.asarray(att_mask)
    qkv_w = np.asarray(qkv_w, dtype=np.float32)
    proj_w = np.asarray(proj_w, dtype=np.float32)
    proj_b = np.asarray(proj_b, dtype=np.float32)

    nc = _get_nc()

    in_maps = []
    for b in range(B):
        xT = np.ascontiguousarray(x[b].T)
        mb = np.where(att_mask[b] == 0, -1e30, 0.0).astype(np.float32)
        mbias = np.ascontiguousarray(mb.reshape(NMT, P).T)
        for g in range(G):
            r0 = g * HPC * DH
            r1 = (g + 1) * HPC * DH
            wq = qkv_w[r0:r1]                # [192, 768]
            wk = qkv_w[C + r0 : C + r1]
            wv = qkv_w[2 * C + r0 : 2 * C + r1]
            wqkT = np.ascontiguousarray(
                np.concatenate(
                    [wq[0 : 2 * DH], wk[0 : 2 * DH], wq[2 * DH :], wk[2 * DH :]], 0
                ).T
            )
            wvT = np.zeros((C, VB), np.float32)
            wvT[:, 0:DH] = wv[0:DH].T              # v h0 at block col 0
            wvT[:, 2 * DH : 3 * DH] = wv[DH : 2 * DH].T   # v h1 at 128:192
            wvT[:, 4 * DH : 5 * DH] = wv[2 * DH :].T      # v h2 at 256:320
            wpT = np.ascontiguousarray(proj_w[:, r0:r1].T)
            in_maps.append(
                {
                    "xT": xT,
                    "wqkT": wqkT,
                    "wvT": wvT,
                    "wpT": wpT,
                    "mbias": mbias,
                    "vones": _VONES,
                }
            )

    res = run_bass_kernel_spmd(
        nc, in_maps, core_ids=list(range(B * G)), trace=TRACE
    )
    LAST_EXEC_NS = res.exec_time_ns
    LAST_RESULTS = res

    out = np.zeros((B, N, C), np.float32)
    for b in range(B):
        acc = res.results[b * G]["y"].copy()
        for g in range(1, G):
            acc += res.results[b * G + g]["y"]
        out[b] = acc + proj_b[None, :]
    return out


# revision 45
# speedup vs baseline: 1.0261x; 1.0079x over previous
"""Multi-head attention forward (B=2, N=2048, C=768, H=12) on 8 TRN2 cores.

Sharding: core = b*4 + g handles batch b, heads 3g..3g+2 (tensor parallel on
heads). Each core computes qkv for its heads, flash-style attention with the
full N x N logits kept on-chip (transposed [m, nq] layout so the key mask
folds into the exp bias and the softmax denominator comes from ones-columns
in V riding the PV matmul), and a partial output projection over its 192
channels. Host sums the 4 partials per batch and adds the bias.

All matmuls run in float32r (TF32-like, 1 cycle/row at free-dim >= 256).
K=64 logits matmuls for head pairs are row-packed onto disjoint PE row
groups (partitions 0:64 vs 64:128) so they run concurrently; head 2 gets
q/k duplicated into both partition halves so its two nq-halves pack the
same way. Head 1's V block is [ones | v] so its attention rows land on
psum partitions 64:128, making the whole normalize chain lane-local and
letting heads 0+1 share one K=128 projection matmul.
"""

import numpy as np

from concourse import bacc
import concourse.mybir as mybir
import concourse.tile as tile
from concourse.bass_utils import run_bass_kernel_spmd

B, N, C = 2, 2048, 768
H, DH = 12, 64
G = 4          # head groups (cores per batch)
HPC = 3        # heads per core
P = 128
KT = C // P    # 6 contraction tiles over channels
NMT = N // P   # 16 key (m) tiles
NQT = N // P   # 16 query tiles
W = 1024       # nq chunk width for logits/exp
NH = N // W    # 2 nq chunks
VBLK = 2 * DH  # 128: per-head lhsT width ([v|1] or [1|v])
VB = 5 * DH    # 320 per m-tile: [v0 | ones | v1 | ones | v2]

TRACE = False
LAST_EXEC_NS = None
LAST_RESULTS = None

_nc_cache = {}

f32 = mybir.dt.float32
f32r = mybir.dt.float32r

_VONES = np.ones((P, NMT * 2 * DH), np.float32)


def _build(reps=1):
    nc = bacc.Bacc("TRN2", debug=False)

    xT = nc.dram_tensor("xT", [C, N], f32r, kind="ExternalInput")
    wqkT = nc.dram_tensor("wqkT", [C, 6 * DH], f32r, kind="ExternalInput")
    wvT = nc.dram_tensor("wvT", [C, VB], f32r, kind="ExternalInput")
    wpT = nc.dram_tensor("wpT", [HPC * DH, C], f32r, kind="ExternalInput")
    mbias = nc.dram_tensor("mbias", [P, NMT], f32, kind="ExternalInput")
    vones = nc.dram_tensor("vones", [P, NMT * 2 * DH], f32r, kind="ExternalInput")
    y = nc.dram_tensor("y", [N, C], f32, kind="ExternalOutput")

    with tile.TileContext(nc) as tc:
        with (
            tc.tile_pool(name="big", bufs=1) as big,
            tc.tile_pool(name="exps", bufs=8) as exps,
            tc.tile_pool(name="recips", bufs=2) as recips,
            tc.tile_pool(name="ys", bufs=6) as ys,
            tc.tile_pool(name="pa", bufs=2, space="PSUM") as pa,
            tc.tile_pool(name="pb", bufs=2, space="PSUM") as pb,
        ):
            body(nc, tc, big, exps, recips, ys, pa, pb,
                 xT, wqkT, wvT, wpT, mbias, vones, y, reps)

    nc.compile()
    return nc


def body(nc, tc, big, exps, recips, ys, pa, pb,
         xT, wqkT, wvT, wpT, mbias, vones, y, reps):
    QC = HPC * DH  # 192: q block width in wqkT
    for _rep in range(reps):
        xT_sb = big.tile([P, KT * N], f32r, tag="xT", name="xT_sb")
        wqk_sb = big.tile([P, KT * 6 * DH], f32r, tag="wqk", name="wqk_sb")
        wv_sb = big.tile([P, KT * VB], f32r, tag="wv", name="wv_sb")
        wpA = big.tile([P, C], f32r, tag="wpA", name="wpA")   # heads 0+1
        wpB = big.tile([P, C], f32r, tag="wpB", name="wpB")   # head 2 rows 64:128
        mb_sb = big.tile([P, NMT], f32, tag="mb", name="mb_sb")
        ones_sb = big.tile([P, P], f32r, tag="ones", name="ones_sb")
        # t1: q/k for heads 0 (parts 0:64) and 1 (parts 64:128)
        # t2: q/k for head 2, duplicated into both partition halves
        t1 = big.tile([P, 2 * N], f32r, tag="t1", name="t1")
        t2 = big.tile([P, 2 * N], f32r, tag="t2", name="t2")
        v_sb = big.tile([P, NMT * VB], f32r, tag="v", name="v_sb")
        atA = big.tile([P, N], f32r, tag="atA", name="atA")   # h0 rows 0:64, h1 rows 64:128
        atB = big.tile([P, N], f32r, tag="atB", name="atB")   # h2 rows 64:128

        # --- input DMAs (coalesced; ordered by first use, xT first) ---
        nc.sync.dma_start(
            wqk_sb[:].rearrange("p (k c) -> p k c", c=6 * DH),
            wqkT[:, :].rearrange("(k p) c -> p k c", p=P),
        )
        xTv = xT[:, :].rearrange("(k p) n -> p k n", p=P)
        xsv = xT_sb[:].rearrange("p (k n) -> p k n", n=N)
        nc.sync.dma_start(xsv[:, 0:2, :], xTv[:, 0:2, :])
        nc.sync.dma_start(xsv[:, 2:4, :], xTv[:, 2:4, :])
        nc.sync.dma_start(xsv[:, 4:6, :], xTv[:, 4:6, :])
        nc.sync.dma_start(
            wv_sb[:].rearrange("p (k c) -> p k c", c=VB),
            wvT[:, :].rearrange("(k p) c -> p k c", p=P),
        )
        nc.sync.dma_start(mb_sb[:], mbias[:, :])
        nc.sync.dma_start(ones_sb[:], vones[:, 0:P])
        # ones regions of v_sb: cols 64:128 and 192:256 of each 320 block
        vview = v_sb[:].rearrange("p (m q) -> p m q", q=VB)
        nc.sync.dma_start(vview[:, :, DH : 2 * DH], vones[:, 0 : NMT * DH])
        nc.sync.dma_start(
            vview[:, :, 3 * DH : 4 * DH], vones[:, NMT * DH : 2 * NMT * DH]
        )
        nc.sync.dma_start(wpA[:], wpT[0 : 2 * DH, :])
        nc.sync.dma_start(wpB[DH:P, :], wpT[2 * DH : 3 * DH, :])

        # --- qT/kT in d-major layout ---
        def qk_pass(c0, copies, chunks=None, pool=None):
            # copies: list of (psum row range, dest tile, dest col offset)
            for ch in chunks if chunks is not None else range(N // W):
                ps = (pool or pa).tile(
                    [P, W], f32, tag="pb" if pool else "pa", name="ps_qk"
                )
                for s in range(W // 512):
                    for k in range(KT):
                        nc.tensor.matmul(
                            ps[:, s * 512 : (s + 1) * 512],
                            wqk_sb[:, k * 6 * DH + c0 : k * 6 * DH + c0 + P],
                            xT_sb[
                                :,
                                k * N + ch * W + s * 512 : k * N
                                + ch * W
                                + (s + 1) * 512,
                            ],
                            start=(k == 0),
                            stop=(k == KT - 1),
                        )
                for r0, r1, dest, dcol in copies:
                    nc.vector.tensor_copy(
                        dest[r0:r1, dcol + ch * W : dcol + (ch + 1) * W],
                        ps[r0:r1, :],
                    )


        # --- v in natural [m, d] layout ---
        def v_tile(mt, pool=None):
            ps = (pool or pb).tile([P, W], f32, tag="pa" if pool else "pb", name="ps_v")
            for k in range(KT):
                nc.tensor.matmul(
                    ps[:, :VB],
                    xT_sb[:, k * N + mt * P : k * N + (mt + 1) * P],
                    wv_sb[:, k * VB : (k + 1) * VB],
                    start=(k == 0),
                    stop=(k == KT - 1),
                )
            for c0 in (0, 2 * DH, 4 * DH):
                nc.vector.tensor_copy(
                    v_sb[:, mt * VB + c0 : mt * VB + c0 + DH],
                    ps[:, c0 : c0 + DH],
                )

        # per-head PV lhsT within m-tile block [v0 |1| v1 |1| v2]:
        # h0 = [v0|ones] (lo), h1 = [ones|v1] (hi), h2 = [ones|v2] (hi)
        def vap(h, mt):
            off = mt * VB + (0 if h == 0 else DH if h == 1 else 3 * DH)
            return v_sb[:, off : off + VBLK]

        # one softmax stream: two row-packed (lo/hi partition) attention
        # pipelines over the 16 key tiles, then lane-local normalize.
        # units: (qk tile, q col, v block col, out AP, flavor)
        # flavor 'lo': v block [v|1] -> attn rows 0:64, s row 64
        # flavor 'hi': v block [1|v] -> s row 0, attn rows 64:128
        def attn_stream(u_lo, u_hi, per_mt=None, tail=False, defer_norm=False):
            ps_lo = pb.tile([P, W], f32, tag="pb", name="ps_pv0")
            ps_hi = pb.tile([P, W], f32, tag="pb", name="ps_pv1")
            for mt in range(NMT):
                if per_mt is not None:
                    per_mt(mt)
                ets = []
                for (qk, qcol, vh, out_ap, flavor), prow in ((u_lo, 0), (u_hi, DH)):
                    ps_l = pa.tile([P, W], f32, tag="pa", name="ps_l")
                    for s in range(W // 512):
                        nc.tensor.matmul(
                            ps_l[:, s * 512 : (s + 1) * 512],
                            qk[prow : prow + DH, N + mt * P : N + (mt + 1) * P],
                            qk[prow : prow + DH, qcol + s * 512 : qcol + (s + 1) * 512],
                            start=True,
                            stop=True,
                        )
                    et = exps.tile([P, W], f32r, tag="exp", name="et")
                    nc.scalar.activation(
                        et[:],
                        ps_l[:],
                        mybir.ActivationFunctionType.Exp,
                        bias=mb_sb[:, mt : mt + 1],
                        scale=float(DH) ** -0.5,
                    )
                    ets.append(et)
                for et, ps_pv, (qk, qcol, vh, out_ap, flavor) in (
                    (ets[0], ps_lo, u_lo),
                    (ets[1], ps_hi, u_hi),
                ):
                    for s in range(W // 512):
                        nc.tensor.matmul(
                            ps_pv[:, s * 512 : (s + 1) * 512],
                            vap(vh, mt),
                            et[:, s * 512 : (s + 1) * 512],
                            start=(mt == 0),
                            stop=(mt == NMT - 1),
                        )
            # normalize (optionally deferred so the next stream's logits
            # claim psum slots before the norm's broadcast tile)
            def normalize():
              for ps_pv, (qk, qcol, vh, out_ap, flavor) in (
                (ps_lo, u_lo),
                (ps_hi, u_hi),
              ):
                rc = recips.tile([P, W], f32r, tag="rc", name="rc")
                ps_rb = pa.tile([P, W], f32, tag="pa", name="ps_rb")
                rb = recips.tile([P, W], f32, tag="rb", name="rb")
                if flavor == "lo":
                    srow, arow = DH, 0
                else:
                    srow, arow = 0, DH
                with nc.allow_low_precision(reason="f32r softmax denom"):
                    nc.vector.reciprocal(
                        rc[srow : srow + 1, :], ps_pv[srow : srow + 1, :]
                    )
                for s in range(W // 512):
                    nc.tensor.matmul(
                        ps_rb[:, s * 512 : (s + 1) * 512],
                        ones_sb[srow : srow + 1, :],
                        rc[srow : srow + 1, s * 512 : (s + 1) * 512],
                        start=True,
                        stop=True,
                    )
                if tail:
                    nc.scalar.copy(
                        rb[arow : arow + DH, :], ps_rb[arow : arow + DH, :]
                    )
                else:
                    nc.vector.tensor_copy(
                        rb[arow : arow + DH, :], ps_rb[arow : arow + DH, :]
                    )
                nc.vector.tensor_mul(
                    out_ap,
                    ps_pv[arow : arow + DH, :],
                    rb[arow : arow + DH, :],
                )

            if defer_norm:
                return normalize
            normalize()
            return None

        def proj(nt):
            ps_y = pa.tile([P, W], f32, tag="pa", name="ps_y")
            for o0, ow in ((0, 512), (512, 256)):
                nc.tensor.matmul(
                    ps_y[:, o0 : o0 + ow],
                    atA[:, nt * P : (nt + 1) * P],
                    wpA[:, o0 : o0 + ow],
                    start=True,
                    stop=False,
                )
                nc.tensor.matmul(
                    ps_y[:, o0 : o0 + ow],
                    atB[DH:P, nt * P : (nt + 1) * P],
                    wpB[DH:P, o0 : o0 + ow],
                    start=False,
                    stop=True,
                )
            yt = ys.tile([P, C], f32, tag="y", name="yt")
            if nt % 2 == 0:
                nc.vector.tensor_copy(yt[:], ps_y[:, :C])
            else:
                nc.scalar.copy(yt[:], ps_y[:, :C])
            nc.sync.dma_start(y[nt * P : (nt + 1) * P, :], yt[:])

        qk_pass(0, [(0, P, t1, 0)])    # q heads 0,1
        qk_pass(P, [(0, P, t1, N)], chunks=[0])          # k heads 0,1
        qk_pass(P, [(0, P, t1, N)], chunks=[1], pool=pb)
        for mt in range(NMT):
            v_tile(mt)
        # q2/k2: rows 0:64 = q2 -> t2 lo/q, rows 64:128 = k2 -> t2 hi/k;
        # the two SBUF->SBUF DMAs fill the other partition halves
        qk_pass(2 * P, [(0, DH, t2, 0), (DH, P, t2, N)], pool=pb)
        nc.sync.dma_start(t2[DH:P, 0:N], t2[0:DH, 0:N])      # q2 -> hi
        nc.sync.dma_start(t2[0:DH, N : 2 * N], t2[DH:P, N : 2 * N])  # k2 -> lo

        # heads 0+1, first nq half (normalize deferred into h2's stream)
        norm0 = attn_stream(
            (t1, 0, 0, atA[0:DH, 0:W], "lo"),
            (t1, 0, 1, atA[DH:P, 0:W], "hi"),
            defer_norm=True,
        )

        # head 2: both nq halves at once via the duplicated q2/k2 rows
        norm2 = attn_stream(
            (t2, 0, 2, atB[DH:P, 0:W], "hi"),
            (t2, W, 2, atB[DH:P, W : 2 * W], "hi"),
            per_mt=lambda mt: norm0() if mt == 2 else None,
            defer_norm=True,
        )

        # heads 0+1 second half: h2's deferred normalize fires early, then
        # half-0's projection tiles interleave so they overlap this stream
        def half1_hook(mt):
            if mt == 2:
                norm2()
            elif 4 <= mt < 4 + NQT // NH:
                proj(mt - 4)

        attn_stream(
            (t1, W, 0, atA[0:DH, W : 2 * W], "lo"),
            (t1, W, 1, atA[DH:P, W : 2 * W], "hi"),
            per_mt=half1_hook,
            tail=True,
        )
        for nt in range(NQT // NH, NQT):
            proj(nt)


def _get_nc(reps=1):
    if reps not in _nc_cache:
        _nc_cache[reps] = _build(reps)
    return _nc_cache[reps]


def kernel(x, att_mask, qkv_w, proj_w, proj_b):
    global LAST_EXEC_NS, LAST_RESULTS
    x = np.asarray(x, dtype=np.float32)
    att_mask = np.asarray(att_mask)
    qkv_w = np.asarray(qkv_w, dtype=np.float32)
    proj_w = np.asarray(proj_w, dtype=np.float32)
    proj_b = np.asarray(proj_b, dtype=np.float32)

    nc = _get_nc()

    in_maps = []
    for b in range(B):
        xT = np.ascontiguousarray(x[b].T)
        mb = np.where(att_mask[b] == 0, -1e30, 0.0).astype(np.float32)
        mbias = np.ascontiguousarray(mb.reshape(NMT, P).T)
        for g in range(G):
            r0 = g * HPC * DH
            r1 = (g + 1) * HPC * DH
            wq = qkv_w[r0:r1]                # [192, 768]
            wk = qkv_w[C + r0 : C + r1]
            wv = qkv_w[2 * C + r0 : 2 * C + r1]
            wqkT = np.ascontiguousarray(
                np.concatenate(
                    [wq[0 : 2 * DH], wk[0 : 2 * DH], wq[2 * DH :], wk[2 * DH :]], 0
                ).T
            )
            wvT = np.zeros((C, VB), np.float32)
            wvT[:, 0:DH] = wv[0:DH].T              # v h0 at block col 0
            wvT[:, 2 * DH : 3 * DH] = wv[DH : 2 * DH].T   # v h1 at 128:192
            wvT[:, 4 * DH : 5 * DH] = wv[2 * DH :].T      # v h2 at 256:320
            wpT = np.ascontiguousarray(proj_w[:, r0:r1].T)
            in_maps.append(
                {
                    "xT": xT,
                    "wqkT": wqkT,
                    "wvT": wvT,
                    "wpT": wpT,
                    "mbias": mbias,
                    "vones": _VONES,
                }
            )

    res = run_bass_kernel_spmd(
        nc, in_maps, core_ids=list(range(B * G)), trace=TRACE
    )
    LAST_EXEC_NS = res.exec_time_ns
    LAST_RESULTS = res

    out = np.zeros((B, N, C), np.float32)
    for b in range(B):
        acc = res.results[b * G]["y"].copy()
        for g in range(1, G):
            acc += res.results[b * G + g]["y"]
        out[b] = acc + proj_b[None, :]
    return out


# revision 48
# speedup vs baseline: 1.0307x; 1.0045x over previous
"""Multi-head attention forward (B=2, N=2048, C=768, H=12) on 8 TRN2 cores.

Sharding: core = b*4 + g handles batch b, heads 3g..3g+2 (tensor parallel on
heads). Each core computes qkv for its heads, flash-style attention with the
full N x N logits kept on-chip (transposed [m, nq] layout so the key mask
folds into the exp bias and the softmax denominator comes from ones-columns
in V riding the PV matmul), and a partial output projection over its 192
channels. Host sums the 4 partials per batch and adds the bias.

All matmuls run in float32r (TF32-like, 1 cycle/row at free-dim >= 256).
K=64 logits matmuls for head pairs are row-packed onto disjoint PE row
groups (partitions 0:64 vs 64:128) so they run concurrently; head 2 gets
q/k duplicated into both partition halves so its two nq-halves pack the
same way. Head 1's V block is [ones | v] so its attention rows land on
psum partitions 64:128, making the whole normalize chain lane-local and
letting heads 0+1 share one K=128 projection matmul.
"""

import numpy as np

from concourse import bacc
import concourse.mybir as mybir
import concourse.tile as tile
from concourse.bass_utils import run_bass_kernel_spmd

B, N, C = 2, 2048, 768
H, DH = 12, 64
G = 4          # head groups (cores per batch)
HPC = 3        # heads per core
P = 128
KT = C // P    # 6 contraction tiles over channels
NMT = N // P   # 16 key (m) tiles
NQT = N // P   # 16 query tiles
W = 1024       # nq chunk width for logits/exp
NH = N // W    # 2 nq chunks
VBLK = 2 * DH  # 128: per-head lhsT width ([v|1] or [1|v])
VB = 5 * DH    # 320 per m-tile: [v0 | ones | v1 | ones | v2]

TRACE = False
LAST_EXEC_NS = None
LAST_RESULTS = None

_nc_cache = {}

f32 = mybir.dt.float32
f32r = mybir.dt.float32r

_VONES = np.ones((P, NMT * 2 * DH), np.float32)


def _build(reps=1):
    nc = bacc.Bacc("TRN2", debug=False)

    xT = nc.dram_tensor("xT", [C, N], f32r, kind="ExternalInput")
    wqkT = nc.dram_tensor("wqkT", [C, 6 * DH], f32r, kind="ExternalInput")
    wvT = nc.dram_tensor("wvT", [C, VB], f32r, kind="ExternalInput")
    wpT = nc.dram_tensor("wpT", [HPC * DH, C], f32r, kind="ExternalInput")
    mbias = nc.dram_tensor("mbias", [P, NMT], f32, kind="ExternalInput")
    vones = nc.dram_tensor("vones", [P, NMT * 2 * DH], f32r, kind="ExternalInput")
    y = nc.dram_tensor("y", [N, C], f32, kind="ExternalOutput")

    with tile.TileContext(nc) as tc:
        with (
            tc.tile_pool(name="big", bufs=1) as big,
            tc.tile_pool(name="exps", bufs=8) as exps,
            tc.tile_pool(name="recips", bufs=2) as recips,
            tc.tile_pool(name="ys", bufs=6) as ys,
            tc.tile_pool(name="pa", bufs=2, space="PSUM") as pa,
            tc.tile_pool(name="pb", bufs=2, space="PSUM") as pb,
        ):
            body(nc, tc, big, exps, recips, ys, pa, pb,
                 xT, wqkT, wvT, wpT, mbias, vones, y, reps)

    nc.compile()
    return nc


def body(nc, tc, big, exps, recips, ys, pa, pb,
         xT, wqkT, wvT, wpT, mbias, vones, y, reps):
    QC = HPC * DH  # 192: q block width in wqkT
    for _rep in range(reps):
        xT_sb = big.tile([P, KT * N], f32r, tag="xT", name="xT_sb")
        wqk_sb = big.tile([P, KT * 6 * DH], f32r, tag="wqk", name="wqk_sb")
        wv_sb = big.tile([P, KT * VB], f32r, tag="wv", name="wv_sb")
        wpA = big.tile([P, C], f32r, tag="wpA", name="wpA")   # heads 0+1
        wpB = big.tile([P, C], f32r, tag="wpB", name="wpB")   # head 2 rows 64:128
        mb_sb = big.tile([P, NMT], f32, tag="mb", name="mb_sb")
        ones_sb = big.tile([P, P], f32r, tag="ones", name="ones_sb")
        # t1: q/k for heads 0 (parts 0:64) and 1 (parts 64:128)
        # t2: q/k for head 2, duplicated into both partition halves
        t1 = big.tile([P, 2 * N], f32r, tag="t1", name="t1")
        t2 = big.tile([P, 2 * N], f32r, tag="t2", name="t2")
        v_sb = big.tile([P, NMT * VB], f32r, tag="v", name="v_sb")
        atA = big.tile([P, N], f32r, tag="atA", name="atA")   # h0 rows 0:64, h1 rows 64:128
        atB = big.tile([P, N], f32r, tag="atB", name="atB")   # h2 rows 64:128

        # --- input DMAs (coalesced; ordered by first use, xT first) ---
        nc.sync.dma_start(
            wqk_sb[:].rearrange("p (k c) -> p k c", c=6 * DH),
            wqkT[:, :].rearrange("(k p) c -> p k c", p=P),
        )
        xTv = xT[:, :].rearrange("(k p) n -> p k n", p=P)
        xsv = xT_sb[:].rearrange("p (k n) -> p k n", n=N)
        nc.sync.dma_start(xsv[:, 0:2, :], xTv[:, 0:2, :])
        nc.sync.dma_start(xsv[:, 2:4, :], xTv[:, 2:4, :])
        nc.sync.dma_start(xsv[:, 4:6, :], xTv[:, 4:6, :])
        nc.sync.dma_start(
            wv_sb[:].rearrange("p (k c) -> p k c", c=VB),
            wvT[:, :].rearrange("(k p) c -> p k c", p=P),
        )
        nc.sync.dma_start(mb_sb[:], mbias[:, :])
        nc.sync.dma_start(ones_sb[:], vones[:, 0:P])
        # ones regions of v_sb: cols 64:128 and 192:256 of each 320 block
        vview = v_sb[:].rearrange("p (m q) -> p m q", q=VB)
        nc.sync.dma_start(vview[:, :, DH : 2 * DH], vones[:, 0 : NMT * DH])
        nc.sync.dma_start(
            vview[:, :, 3 * DH : 4 * DH], vones[:, NMT * DH : 2 * NMT * DH]
        )
        nc.sync.dma_start(wpA[:], wpT[0 : 2 * DH, :])
        nc.sync.dma_start(wpB[DH:P, :], wpT[2 * DH : 3 * DH, :])

        # --- qT/kT in d-major layout ---
        def qk_pass(c0, copies, chunks=None, pool=None):
            # copies: list of (psum row range, dest tile, dest col offset)
            for ch in chunks if chunks is not None else range(N // W):
                ps = (pool or pa).tile(
                    [P, W], f32, tag="pb" if pool else "pa", name="ps_qk"
                )
                for s in range(W // 512):
                    for k in range(KT):
                        nc.tensor.matmul(
                            ps[:, s * 512 : (s + 1) * 512],
                            wqk_sb[:, k * 6 * DH + c0 : k * 6 * DH + c0 + P],
                            xT_sb[
                                :,
                                k * N + ch * W + s * 512 : k * N
                                + ch * W
                                + (s + 1) * 512,
                            ],
                            start=(k == 0),
                            stop=(k == KT - 1),
                        )
                for r0, r1, dest, dcol in copies:
                    nc.vector.tensor_copy(
                        dest[r0:r1, dcol + ch * W : dcol + (ch + 1) * W],
                        ps[r0:r1, :],
                    )


        # --- v in natural [m, d] layout ---
        def v_tile(mt, pool=None):
            ps = (pool or pb).tile([P, W], f32, tag="pa" if pool else "pb", name="ps_v")
            for k in range(KT):
                nc.tensor.matmul(
                    ps[:, :VB],
                    xT_sb[:, k * N + mt * P : k * N + (mt + 1) * P],
                    wv_sb[:, k * VB : (k + 1) * VB],
                    start=(k == 0),
                    stop=(k == KT - 1),
                )
            for c0 in (0, 2 * DH, 4 * DH):
                nc.vector.tensor_copy(
                    v_sb[:, mt * VB + c0 : mt * VB + c0 + DH],
                    ps[:, c0 : c0 + DH],
                )

        # per-head PV lhsT within m-tile block [v0 |1| v1 |1| v2]:
        # h0 = [v0|ones] (lo), h1 = [ones|v1] (hi), h2 = [ones|v2] (hi)
        def vap(h, mt):
            off = mt * VB + (0 if h == 0 else DH if h == 1 else 3 * DH)
            return v_sb[:, off : off + VBLK]

        # one softmax stream: two row-packed (lo/hi partition) attention
        # pipelines over the 16 key tiles, then lane-local normalize.
        # units: (qk tile, q col, v block col, out AP, flavor)
        # flavor 'lo': v block [v|1] -> attn rows 0:64, s row 64
        # flavor 'hi': v block [1|v] -> s row 0, attn rows 64:128
        def attn_stream(u_lo, u_hi, per_mt=None, tail=False, defer_norm=False):
            ps_lo = pb.tile([P, W], f32, tag="pb", name="ps_pv0")
            ps_hi = pb.tile([P, W], f32, tag="pb", name="ps_pv1")
            for mt in range(NMT):
                if per_mt is not None:
                    per_mt(mt)
                ets = []
                for (qk, qcol, vh, out_ap, flavor), prow in ((u_lo, 0), (u_hi, DH)):
                    ps_l = pa.tile([P, W], f32, tag="pa", name="ps_l")
                    for s in range(W // 512):
                        nc.tensor.matmul(
                            ps_l[:, s * 512 : (s + 1) * 512],
                            qk[prow : prow + DH, N + mt * P : N + (mt + 1) * P],
                            qk[prow : prow + DH, qcol + s * 512 : qcol + (s + 1) * 512],
                            start=True,
                            stop=True,
                        )
                    et = exps.tile([P, W], f32r, tag="exp", name="et")
                    nc.scalar.activation(
                        et[:],
                        ps_l[:],
                        mybir.ActivationFunctionType.Exp,
                        bias=mb_sb[:, mt : mt + 1],
                        scale=float(DH) ** -0.5,
                    )
                    ets.append(et)
                for et, ps_pv, (qk, qcol, vh, out_ap, flavor) in (
                    (ets[0], ps_lo, u_lo),
                    (ets[1], ps_hi, u_hi),
                ):
                    for s in range(W // 512):
                        nc.tensor.matmul(
                            ps_pv[:, s * 512 : (s + 1) * 512],
                            vap(vh, mt),
                            et[:, s * 512 : (s + 1) * 512],
                            start=(mt == 0),
                            stop=(mt == NMT - 1),
                        )
            # normalize (optionally deferred so the next stream's logits
            # claim psum slots before the norm's broadcast tile)
            def normalize():
              for ps_pv, (qk, qcol, vh, out_ap, flavor) in (
                (ps_lo, u_lo),
                (ps_hi, u_hi),
              ):
                rc = recips.tile([P, W], f32r, tag="rc", name="rc")
                ps_rb = pa.tile([P, W], f32, tag="pa", name="ps_rb")
                rb = recips.tile([P, W], f32, tag="rb", name="rb")
                if flavor == "lo":
                    srow, arow = DH, 0
                else:
                    srow, arow = 0, DH
                with nc.allow_low_precision(reason="f32r softmax denom"):
                    nc.vector.reciprocal(
                        rc[srow : srow + 1, :], ps_pv[srow : srow + 1, :]
                    )
                for s in range(W // 512):
                    nc.tensor.matmul(
                        ps_rb[:, s * 512 : (s + 1) * 512],
                        ones_sb[srow : srow + 1, :],
                        rc[srow : srow + 1, s * 512 : (s + 1) * 512],
                        start=True,
                        stop=True,
                    )
                if tail:
                    nc.scalar.copy(
                        rb[arow : arow + DH, :], ps_rb[arow : arow + DH, :]
                    )
                else:
                    nc.vector.tensor_copy(
                        rb[arow : arow + DH, :], ps_rb[arow : arow + DH, :]
                    )
                nc.vector.tensor_mul(
                    out_ap,
                    ps_pv[arow : arow + DH, :],
                    rb[arow : arow + DH, :],
                )

            if defer_norm:
                return normalize
            normalize()
            return None

        def proj(nt):
            ps_y = pa.tile([P, W], f32, tag="pa", name="ps_y")
            for o0, ow in ((0, 512), (512, 256)):
                nc.tensor.matmul(
                    ps_y[:, o0 : o0 + ow],
                    atA[:, nt * P : (nt + 1) * P],
                    wpA[:, o0 : o0 + ow],
                    start=True,
                    stop=False,
                )
                nc.tensor.matmul(
                    ps_y[:, o0 : o0 + ow],
                    atB[DH:P, nt * P : (nt + 1) * P],
                    wpB[DH:P, o0 : o0 + ow],
                    start=False,
                    stop=True,
                )
            yt = ys.tile([P, C], f32, tag="y", name="yt")
            if nt % 2 == 0:
                nc.vector.tensor_copy(yt[:], ps_y[:, :C])
            else:
                nc.scalar.copy(yt[:], ps_y[:, :C])
            nc.sync.dma_start(y[nt * P : (nt + 1) * P, :], yt[:])

        qk_pass(0, [(0, P, t1, 0)], chunks=[0])          # q heads 0,1
        qk_pass(0, [(0, P, t1, 0)], chunks=[1], pool=pb)
        qk_pass(P, [(0, P, t1, N)], chunks=[0])          # k heads 0,1
        qk_pass(P, [(0, P, t1, N)], chunks=[1], pool=pb)
        for mt in range(NMT):
            v_tile(mt)
        # q2/k2: rows 0:64 = q2 -> t2 lo/q, rows 64:128 = k2 -> t2 hi/k;
        # the two SBUF->SBUF DMAs fill the other partition halves
        qk_pass(2 * P, [(0, DH, t2, 0), (DH, P, t2, N)], pool=pb)
        nc.sync.dma_start(t2[DH:P, 0:N], t2[0:DH, 0:N])      # q2 -> hi
        nc.sync.dma_start(t2[0:DH, N : 2 * N], t2[DH:P, N : 2 * N])  # k2 -> lo

        # heads 0+1, first nq half (normalize deferred into h2's stream)
        norm0 = attn_stream(
            (t1, 0, 0, atA[0:DH, 0:W], "lo"),
            (t1, 0, 1, atA[DH:P, 0:W], "hi"),
            defer_norm=True,
        )

        # head 2: both nq halves at once via the duplicated q2/k2 rows
        norm2 = attn_stream(
            (t2, 0, 2, atB[DH:P, 0:W], "hi"),
            (t2, W, 2, atB[DH:P, W : 2 * W], "hi"),
            per_mt=lambda mt: norm0() if mt == 2 else None,
            defer_norm=True,
        )

        # heads 0+1 second half: h2's deferred normalize fires early, then
        # half-0's projection tiles interleave so they overlap this stream
        def half1_hook(mt):
            if mt == 2:
                norm2()
            elif 4 <= mt < 4 + NQT // NH:
                proj(mt - 4)

        attn_stream(
            (t1, W, 0, atA[0:DH, W : 2 * W], "lo"),
            (t1, W, 1, atA[DH:P, W : 2 * W], "hi"),
            per_mt=half1_hook,
            tail=True,
        )
        for nt in range(NQT // NH, NQT):
            proj(nt)


def _get_nc(reps=1):
    if reps not in _nc_cache:
        _nc_cache[reps] = _build(reps)
    return _nc_cache[reps]


def kernel(x, att_mask, qkv_w, proj_w, proj_b):
    global LAST_EXEC_NS, LAST_RESULTS
    x = np.asarray(x, dtype=np.float32)
    att_mask = np.asarray(att_mask)
    qkv_w = np.asarray(qkv_w, dtype=np.float32)
    proj_w = np.asarray(proj_w, dtype=np.float32)
    proj_b = np.asarray(proj_b, dtype=np.float32)

    nc = _get_nc()

    in_maps = []
    for b in range(B):
        xT = np.ascontiguousarray(x[b].T)
        mb = np.where(att_mask[b] == 0, -1e30, 0.0).astype(np.float32)
        mbias = np.ascontiguousarray(mb.reshape(NMT, P).T)
        for g in range(G):
            r0 = g * HPC * DH
            r1 = (g + 1) * HPC * DH
            wq = qkv_w[r0:r1]                # [192, 768]
            wk = qkv_w[C + r0 : C + r1]
            wv = qkv_w[2 * C + r0 : 2 * C + r1]
            wqkT = np.ascontiguousarray(
                np.concatenate(
                    [wq[0 : 2 * DH], wk[0 : 2 * DH], wq[2 * DH :], wk[2 * DH :]], 0
                ).T
            )
            wvT = np.zeros((C, VB), np.float32)
            wvT[:, 0:DH] = wv[0:DH].T              # v h0 at block col 0
            wvT[:, 2 * DH : 3 * DH] = wv[DH : 2 * DH].T   # v h1 at 128:192
            wvT[:, 4 * DH : 5 * DH] = wv[2 * DH :].T      # v h2 at 256:320
            wpT = np.ascontiguousarray(proj_w[:, r0:r1].T)
            in_maps.append(
                {
                    "xT": xT,
                    "wqkT": wqkT,
                    "wvT": wvT,
                    "wpT": wpT,
                    "mbias": mbias,
                    "vones": _VONES,
                }
            )

    res = run_bass_kernel_spmd(
        nc, in_maps, core_ids=list(range(B * G)), trace=TRACE
    )
    LAST_EXEC_NS = res.exec_time_ns
    LAST_RESULTS = res

    out = np.zeros((B, N, C), np.float32)
    for b in range(B):
        acc = res.results[b * G]["y"].copy()
        for g in range(1, G):
            acc += res.results[b * G + g]["y"]
        out[b] = acc + proj_b[None, :]
    return out


# revision 49
# speedup vs baseline: 1.0347x; 1.0038x over previous
"""Multi-head attention forward (B=2, N=2048, C=768, H=12) on 8 TRN2 cores.

Sharding: core = b*4 + g handles batch b, heads 3g..3g+2 (tensor parallel on
heads). Each core computes qkv for its heads, flash-style attention with the
full N x N logits kept on-chip (transposed [m, nq] layout so the key mask
folds into the exp bias and the softmax denominator comes from ones-columns
in V riding the PV matmul), and a partial output projection over its 192
channels. Host sums the 4 partials per batch and adds the bias.

All matmuls run in float32r (TF32-like, 1 cycle/row at free-dim >= 256).
K=64 logits matmuls for head pairs are row-packed onto disjoint PE row
groups (partitions 0:64 vs 64:128) so they run concurrently; head 2 gets
q/k duplicated into both partition halves so its two nq-halves pack the
same way. Head 1's V block is [ones | v] so its attention rows land on
psum partitions 64:128, making the whole normalize chain lane-local and
letting heads 0+1 share one K=128 projection matmul.
"""

import numpy as np

from concourse import bacc
import concourse.mybir as mybir
import concourse.tile as tile
from concourse.bass_utils import run_bass_kernel_spmd

B, N, C = 2, 2048, 768
H, DH = 12, 64
G = 4          # head groups (cores per batch)
HPC = 3        # heads per core
P = 128
KT = C // P    # 6 contraction tiles over channels
NMT = N // P   # 16 key (m) tiles
NQT = N // P   # 16 query tiles
W = 1024       # nq chunk width for logits/exp
NH = N // W    # 2 nq chunks
VBLK = 2 * DH  # 128: per-head lhsT width ([v|1] or [1|v])
VB = 5 * DH    # 320 per m-tile: [v0 | ones | v1 | ones | v2]

TRACE = False
LAST_EXEC_NS = None
LAST_RESULTS = None

_nc_cache = {}

f32 = mybir.dt.float32
f32r = mybir.dt.float32r

_VONES = np.ones((P, NMT * 2 * DH), np.float32)


def _build(reps=1):
    nc = bacc.Bacc("TRN2", debug=False)

    xT = nc.dram_tensor("xT", [C, N], f32r, kind="ExternalInput")
    wqkT = nc.dram_tensor("wqkT", [C, 6 * DH], f32r, kind="ExternalInput")
    wvT = nc.dram_tensor("wvT", [C, VB], f32r, kind="ExternalInput")
    wpT = nc.dram_tensor("wpT", [HPC * DH, C], f32r, kind="ExternalInput")
    mbias = nc.dram_tensor("mbias", [P, NMT], f32, kind="ExternalInput")
    vones = nc.dram_tensor("vones", [P, NMT * 2 * DH], f32r, kind="ExternalInput")
    y = nc.dram_tensor("y", [N, C], f32, kind="ExternalOutput")

    with tile.TileContext(nc) as tc:
        with (
            tc.tile_pool(name="big", bufs=1) as big,
            tc.tile_pool(name="exps", bufs=8) as exps,
            tc.tile_pool(name="recips", bufs=2) as recips,
            tc.tile_pool(name="ys", bufs=6) as ys,
            tc.tile_pool(name="pa", bufs=2, space="PSUM") as pa,
            tc.tile_pool(name="pb", bufs=2, space="PSUM") as pb,
        ):
            body(nc, tc, big, exps, recips, ys, pa, pb,
                 xT, wqkT, wvT, wpT, mbias, vones, y, reps)

    nc.compile()
    return nc


def body(nc, tc, big, exps, recips, ys, pa, pb,
         xT, wqkT, wvT, wpT, mbias, vones, y, reps):
    QC = HPC * DH  # 192: q block width in wqkT
    for _rep in range(reps):
        xT_sb = big.tile([P, KT * N], f32r, tag="xT", name="xT_sb")
        wqk_sb = big.tile([P, KT * 6 * DH], f32r, tag="wqk", name="wqk_sb")
        wv_sb = big.tile([P, KT * VB], f32r, tag="wv", name="wv_sb")
        wpA = big.tile([P, C], f32r, tag="wpA", name="wpA")   # heads 0+1
        wpB = big.tile([P, C], f32r, tag="wpB", name="wpB")   # head 2 rows 64:128
        mb_sb = big.tile([P, NMT], f32, tag="mb", name="mb_sb")
        ones_sb = big.tile([P, P], f32r, tag="ones", name="ones_sb")
        # t1: q/k for heads 0 (parts 0:64) and 1 (parts 64:128)
        # t2: q/k for head 2, duplicated into both partition halves
        t1 = big.tile([P, 2 * N], f32r, tag="t1", name="t1")
        t2 = big.tile([P, 2 * N], f32r, tag="t2", name="t2")
        v_sb = big.tile([P, NMT * VB], f32r, tag="v", name="v_sb")
        atA = big.tile([P, N], f32r, tag="atA", name="atA")   # h0 rows 0:64, h1 rows 64:128
        atB = big.tile([P, N], f32r, tag="atB", name="atB")   # h2 rows 64:128

        # --- input DMAs (coalesced; ordered by first use, xT first) ---
        nc.sync.dma_start(
            wqk_sb[:].rearrange("p (k c) -> p k c", c=6 * DH),
            wqkT[:, :].rearrange("(k p) c -> p k c", p=P),
        )
        xTv = xT[:, :].rearrange("(k p) n -> p k n", p=P)
        xsv = xT_sb[:].rearrange("p (k n) -> p k n", n=N)
        nc.sync.dma_start(xsv[:, 0:2, :], xTv[:, 0:2, :])
        nc.sync.dma_start(xsv[:, 2:4, :], xTv[:, 2:4, :])
        nc.sync.dma_start(xsv[:, 4:6, :], xTv[:, 4:6, :])
        nc.sync.dma_start(
            wv_sb[:].rearrange("p (k c) -> p k c", c=VB),
            wvT[:, :].rearrange("(k p) c -> p k c", p=P),
        )
        nc.sync.dma_start(mb_sb[:], mbias[:, :])
        nc.sync.dma_start(ones_sb[:], vones[:, 0:P])
        # ones regions of v_sb: cols 64:128 and 192:256 of each 320 block
        vview = v_sb[:].rearrange("p (m q) -> p m q", q=VB)
        nc.sync.dma_start(vview[:, :, DH : 2 * DH], vones[:, 0 : NMT * DH])
        nc.sync.dma_start(
            vview[:, :, 3 * DH : 4 * DH], vones[:, NMT * DH : 2 * NMT * DH]
        )
        nc.sync.dma_start(wpA[:], wpT[0 : 2 * DH, :])
        nc.sync.dma_start(wpB[DH:P, :], wpT[2 * DH : 3 * DH, :])

        # --- qT/kT in d-major layout ---
        def qk_pass(c0, copies, chunks=None, pool=None):
            # copies: list of (psum row range, dest tile, dest col offset)
            for ch in chunks if chunks is not None else range(N // W):
                ps = (pool or pa).tile(
                    [P, W], f32, tag="pb" if pool else "pa", name="ps_qk"
                )
                for s in range(W // 512):
                    for k in range(KT):
                        nc.tensor.matmul(
                            ps[:, s * 512 : (s + 1) * 512],
                            wqk_sb[:, k * 6 * DH + c0 : k * 6 * DH + c0 + P],
                            xT_sb[
                                :,
                                k * N + ch * W + s * 512 : k * N
                                + ch * W
                                + (s + 1) * 512,
                            ],
                            start=(k == 0),
                            stop=(k == KT - 1),
                        )
                for r0, r1, dest, dcol in copies:
                    nc.vector.tensor_copy(
                        dest[r0:r1, dcol + ch * W : dcol + (ch + 1) * W],
                        ps[r0:r1, :],
                    )


        # --- v in natural [m, d] layout ---
        def v_tile(mt, pool=None):
            ps = (pool or pb).tile([P, W], f32, tag="pa" if pool else "pb", name="ps_v")
            for k in range(KT):
                nc.tensor.matmul(
                    ps[:, :VB],
                    xT_sb[:, k * N + mt * P : k * N + (mt + 1) * P],
                    wv_sb[:, k * VB : (k + 1) * VB],
                    start=(k == 0),
                    stop=(k == KT - 1),
                )
            for c0 in (0, 2 * DH, 4 * DH):
                nc.vector.tensor_copy(
                    v_sb[:, mt * VB + c0 : mt * VB + c0 + DH],
                    ps[:, c0 : c0 + DH],
                )

        # per-head PV lhsT within m-tile block [v0 |1| v1 |1| v2]:
        # h0 = [v0|ones] (lo), h1 = [ones|v1] (hi), h2 = [ones|v2] (hi)
        def vap(h, mt):
            off = mt * VB + (0 if h == 0 else DH if h == 1 else 3 * DH)
            return v_sb[:, off : off + VBLK]

        # one softmax stream: two row-packed (lo/hi partition) attention
        # pipelines over the 16 key tiles, then lane-local normalize.
        # units: (qk tile, q col, v block col, out AP, flavor)
        # flavor 'lo': v block [v|1] -> attn rows 0:64, s row 64
        # flavor 'hi': v block [1|v] -> s row 0, attn rows 64:128
        def attn_stream(u_lo, u_hi, per_mt=None, tail=False, defer_norm=False):
            ps_lo = pb.tile([P, W], f32, tag="pb", name="ps_pv0")
            ps_hi = pb.tile([P, W], f32, tag="pb", name="ps_pv1")
            for mt in range(NMT):
                if per_mt is not None:
                    per_mt(mt)
                ets = []
                for (qk, qcol, vh, out_ap, flavor), prow in ((u_lo, 0), (u_hi, DH)):
                    ps_l = pa.tile([P, W], f32, tag="pa", name="ps_l")
                    for s in range(W // 512):
                        nc.tensor.matmul(
                            ps_l[:, s * 512 : (s + 1) * 512],
                            qk[prow : prow + DH, N + mt * P : N + (mt + 1) * P],
                            qk[prow : prow + DH, qcol + s * 512 : qcol + (s + 1) * 512],
                            start=True,
                            stop=True,
                        )
                    et = exps.tile([P, W], f32r, tag="exp", name="et")
                    nc.scalar.activation(
                        et[:],
                        ps_l[:],
                        mybir.ActivationFunctionType.Exp,
                        bias=mb_sb[:, mt : mt + 1],
                        scale=float(DH) ** -0.5,
                    )
                    ets.append(et)
                for et, ps_pv, (qk, qcol, vh, out_ap, flavor) in (
                    (ets[0], ps_lo, u_lo),
                    (ets[1], ps_hi, u_hi),
                ):
                    for s in range(W // 512):
                        nc.tensor.matmul(
                            ps_pv[:, s * 512 : (s + 1) * 512],
                            vap(vh, mt),
                            et[:, s * 512 : (s + 1) * 512],
                            start=(mt == 0),
                            stop=(mt == NMT - 1),
                        )
            # normalize (optionally deferred so the next stream's logits
            # claim psum slots before the norm's broadcast tile)
            def normalize():
              for ps_pv, (qk, qcol, vh, out_ap, flavor) in (
                (ps_lo, u_lo),
                (ps_hi, u_hi),
              ):
                rc = recips.tile([P, W], f32r, tag="rc", name="rc")
                ps_rb = pa.tile([P, W], f32, tag="pa", name="ps_rb")
                rb = recips.tile([P, W], f32, tag="rb", name="rb")
                if flavor == "lo":
                    srow, arow = DH, 0
                else:
                    srow, arow = 0, DH
                with nc.allow_low_precision(reason="f32r softmax denom"):
                    nc.vector.reciprocal(
                        rc[srow : srow + 1, :], ps_pv[srow : srow + 1, :]
                    )
                for s in range(W // 512):
                    nc.tensor.matmul(
                        ps_rb[:, s * 512 : (s + 1) * 512],
                        ones_sb[srow : srow + 1, :],
                        rc[srow : srow + 1, s * 512 : (s + 1) * 512],
                        start=True,
                        stop=True,
                    )
                if tail:
                    nc.scalar.copy(
                        rb[arow : arow + DH, :], ps_rb[arow : arow + DH, :]
                    )
                else:
                    nc.vector.tensor_copy(
                        rb[arow : arow + DH, :], ps_rb[arow : arow + DH, :]
                    )
                nc.vector.tensor_mul(
                    out_ap,
                    ps_pv[arow : arow + DH, :],
                    rb[arow : arow + DH, :],
                )

            if defer_norm:
                return normalize
            normalize()
            return None

        def proj(nt):
            ps_y = pa.tile([P, W], f32, tag="pa", name="ps_y")
            for o0, ow in ((0, 512), (512, 256)):
                nc.tensor.matmul(
                    ps_y[:, o0 : o0 + ow],
                    atA[:, nt * P : (nt + 1) * P],
                    wpA[:, o0 : o0 + ow],
                    start=True,
                    stop=False,
                )
                nc.tensor.matmul(
                    ps_y[:, o0 : o0 + ow],
                    atB[DH:P, nt * P : (nt + 1) * P],
                    wpB[DH:P, o0 : o0 + ow],
                    start=False,
                    stop=True,
                )
            yt = ys.tile([P, C], f32, tag="y", name="yt")
            if nt % 2 == 0:
                nc.vector.tensor_copy(yt[:], ps_y[:, :C])
            else:
                nc.scalar.copy(yt[:], ps_y[:, :C])
            nc.sync.dma_start(y[nt * P : (nt + 1) * P, :], yt[:])

        qk_pass(0, [(0, P, t1, 0)], chunks=[0])          # q heads 0,1
        qk_pass(0, [(0, P, t1, 0)], chunks=[1], pool=pb)
        qk_pass(P, [(0, P, t1, N)], chunks=[0])          # k heads 0,1
        qk_pass(P, [(0, P, t1, N)], chunks=[1], pool=pb)
        for mt in range(NMT):
            v_tile(mt)
        # q2/k2: rows 0:64 = q2 -> t2 lo/q, rows 64:128 = k2 -> t2 hi/k;
        # the two SBUF->SBUF DMAs fill the other partition halves
        qk_pass(2 * P, [(0, DH, t2, 0), (DH, P, t2, N)], pool=pb)
        nc.sync.dma_start(t2[DH:P, 0:N], t2[0:DH, 0:N])      # q2 -> hi
        nc.sync.dma_start(t2[0:DH, N : 2 * N], t2[DH:P, N : 2 * N])  # k2 -> lo

        # heads 0+1, first nq half (normalize deferred into h2's stream)
        norm0 = attn_stream(
            (t1, 0, 0, atA[0:DH, 0:W], "lo"),
            (t1, 0, 1, atA[DH:P, 0:W], "hi"),
            defer_norm=True,
        )

        # head 2: both nq halves at once via the duplicated q2/k2 rows
        norm2 = attn_stream(
            (t2, 0, 2, atB[DH:P, 0:W], "hi"),
            (t2, W, 2, atB[DH:P, W : 2 * W], "hi"),
            per_mt=lambda mt: norm0() if mt == 1 else None,
            defer_norm=True,
        )

        # heads 0+1 second half: h2's deferred normalize fires early, then
        # half-0's projection tiles interleave so they overlap this stream
        def half1_hook(mt):
            if mt == 1:
                norm2()
            elif 4 <= mt < 4 + NQT // NH:
                proj(mt - 4)

        attn_stream(
            (t1, W, 0, atA[0:DH, W : 2 * W], "lo"),
            (t1, W, 1, atA[DH:P, W : 2 * W], "hi"),
            per_mt=half1_hook,
            tail=True,
        )
        for nt in range(NQT // NH, NQT):
            proj(nt)


def _get_nc(reps=1):
    if reps not in _nc_cache:
        _nc_cache[reps] = _build(reps)
    return _nc_cache[reps]


def kernel(x, att_mask, qkv_w, proj_w, proj_b):
    global LAST_EXEC_NS, LAST_RESULTS
    x = np.asarray(x, dtype=np.float32)
    att_mask = np.asarray(att_mask)
    qkv_w = np.asarray(qkv_w, dtype=np.float32)
    proj_w = np.asarray(proj_w, dtype=np.float32)
    proj_b = np.asarray(proj_b, dtype=np.float32)

    nc = _get_nc()

    in_maps = []
    for b in range(B):
        xT = np.ascontiguousarray(x[b].T)
        mb = np.where(att_mask[b] == 0, -1e30, 0.0).astype(np.float32)
        mbias = np.ascontiguousarray(mb.reshape(NMT, P).T)
        for g in range(G):
            r0 = g * HPC * DH
            r1 = (g + 1) * HPC * DH
            wq = qkv_w[r0:r1]                # [192, 768]
            wk = qkv_w[C + r0 : C + r1]
            wv = qkv_w[2 * C + r0 : 2 * C + r1]
            wqkT = np.ascontiguousarray(
                np.concatenate(
                    [wq[0 : 2 * DH], wk[0 : 2 * DH], wq[2 * DH :], wk[2 * DH :]], 0
                ).T
            )
            wvT = np.zeros((C, VB), np.float32)
            wvT[:, 0:DH] = wv[0:DH].T              # v h0 at block col 0
            wvT[:, 2 * DH : 3 * DH] = wv[DH : 2 * DH].T   # v h1 at 128:192
            wvT[:, 4 * DH : 5 * DH] = wv[2 * DH :].T      # v h2 at 256:320
            wpT = np.ascontiguousarray(proj_w[:, r0:r1].T)
            in_maps.append(
                {
                    "xT": xT,
                    "wqkT": wqkT,
                    "wvT": wvT,
                    "wpT": wpT,
                    "mbias": mbias,
                    "vones": _VONES,
                }
            )

    res = run_bass_kernel_spmd(
        nc, in_maps, core_ids=list(range(B * G)), trace=TRACE
    )
    LAST_EXEC_NS = res.exec_time_ns
    LAST_RESULTS = res

    out = np.zeros((B, N, C), np.float32)
    for b in range(B):
        acc = res.results[b * G]["y"].copy()
        for g in range(1, G):
            acc += res.results[b * G + g]["y"]
        out[b] = acc + proj_b[None, :]
    return out
